# revision 1
# baseline (speedup 1.0000x reference)
"""CrystalGraphALIGNN Trainium2 kernel (8 NeuronCores, SPMD).

Strategy: dst-shard edges across cores (atom v owned by core v // (N/8); edge
(i,j) owned by the core of its dst). Per core, edges are sorted by dst and
grouped into 128-atom blocks so that:
  - the dst-side expansion A_dst[dst(e)] is a block-local one-hot matmul,
  - the scatter-mean aggregation is a one-hot matmul into PSUM,
  - only the src side needs a true random gather: per-edge rows of
    A_src = node @ W_src, fetched with dma_gather (transposed, bf16) from a
    DRAM table that is refreshed once per layer via AllGather.
The block-local one-hot matrices are never shipped from the host: the host
sends one bf16/f32 row index per edge slot and the kernel expands them into
a DRAM scratch at init with a rank-1 broadcast matmul + is_equal compare.
Node states and the node MLP stay fully shard-local; crystal pooling is a
one-hot matmul (also built on device from per-atom crystal ids) + a single
AllReduce, readout replicated on every core.
"""

import numpy as np
import ml_dtypes

import concourse.bass as bass
import concourse.bacc as bacc
import concourse.mybir as mybir
import concourse.tile as tile
from concourse import library_config

F32 = mybir.dt.float32
BF16 = mybir.dt.bfloat16
F8 = mybir.dt.float8e4
I16 = mybir.dt.int16
AFT = mybir.ActivationFunctionType
ALU = mybir.AluOpType
BF = ml_dtypes.bfloat16
F8E4 = ml_dtypes.float8_e4m3

NCORES = 8
ED, ND, HID, RD = 64, 128, 128, 128
EDGE_THRESH = 1e-6
GC = 125  # crystals per pooling group

FULL_CFG = dict(N=50000, M=12, AFD=92, EFD=41, NCRYS=1000, L=4)


def _cdiv(a, b):
    return (a + b - 1) // b


def _wrap_idx(flat):
    """int16 flat idx [n] -> [16, n/16] wrapped layout (replicated on device)."""
    n = flat.shape[-1]
    assert n % 16 == 0
    return np.ascontiguousarray(flat.reshape(n // 16, 16).T.astype(np.int16))


def _prep(inputs, cfg):
    N, M, AFD, EFD, NCRYS, L = (cfg[k] for k in ("N", "M", "AFD", "EFD", "NCRYS", "L"))
    ASH = N // NCORES
    NBLK = _cdiv(ASH, 128)
    LOS = min(25000, N)  # src index split for int16 gather indices
    NG = _cdiv(NCRYS, GC)

    af = np.asarray(inputs["atom_fea"], np.float32)
    nf = np.asarray(inputs["nbr_fea"], np.float32)
    nidx = np.asarray(inputs["nbr_fea_idx"]).astype(np.int64)
    cb = np.asarray(inputs["crystal_batch"]).astype(np.int64)

    E = N * M
    dst = np.clip(nidx.reshape(-1), 0, N - 1)
    src = np.repeat(np.arange(N, dtype=np.int64), M)
    ea = nf.reshape(E, EFD)
    mask = (np.abs(ea).sum(1) > EDGE_THRESH).astype(np.float32)

    cnt = np.bincount(dst, weights=mask, minlength=N)
    invcnt = (1.0 / np.maximum(cnt, 1.0)).astype(np.float32)
    ccnt = np.bincount(cb, minlength=NCRYS).astype(np.float32)
    invccnt = (1.0 / np.maximum(ccnt, 1.0)).astype(np.float32)

    core_of = dst // ASH
    dloc = dst - core_of * ASH
    blk_of = dloc // 128
    arow = dloc - blk_of * 128
    half = (src >= LOS).astype(np.int64)

    # per-(core, block, half) edge counts -> shared tile geometry
    key = (core_of * NBLK + blk_of) * 2 + half
    cnts = np.bincount(key, minlength=NCORES * NBLK * 2).reshape(NCORES, NBLK, 2)
    T_lo = np.maximum((cnts[:, :, 0].max(0) + 127) // 128, 1)
    T_hi = (cnts[:, :, 1].max(0) + 127) // 128
    odd = ((T_lo + T_hi) % 2).astype(np.int64)
    if N > LOS:
        T_hi = T_hi + odd
    else:
        T_lo = T_lo + odd
    n_lo = T_lo * 128
    n_hi = T_hi * 128
    nblk_e = n_lo + n_hi
    ecol = np.zeros(NBLK + 1, np.int64)
    ecol[1:] = np.cumsum(nblk_e)
    EP = int(ecol[NBLK])

    # geometry: edge-col space (block-major), state-col space (per half),
    # chunk list entries: (state_col, edge_col, blk_edge_col, n)
    BHALF = NBLK // 2
    scol = np.zeros(NBLK, np.int64)
    acc = [0, 0]
    blocks = []
    for b in range(NBLK):
        hb = 0 if b < BHALF else 1
        scol[b] = acc[hb]
        acc[hb] += int(nblk_e[b])
        tiles = int(T_lo[b] + T_hi[b])
        chunks = []
        off = 0
        while tiles > 0:
            t = 4 if tiles >= 4 else tiles
            chunks.append((int(scol[b] + off), int(ecol[b] + off), off, t * 128))
            off += t * 128
            tiles -= t
        blocks.append(dict(b=b, half=hb, nblk_e=int(nblk_e[b]), chunks=chunks,
                           n_lo=int(n_lo[b]), n_hi=int(n_hi[b])))
    EPC = max(acc)
    IWL = int(T_lo.sum()) * 8
    IWH = int(T_hi.sum()) * 8

    meta = dict(cfg=cfg, ASH=ASH, NBLK=NBLK, LOS=LOS, NG=NG, EP=EP, EPC=EPC,
                BHALF=BHALF, blocks=blocks, IWL=IWL, IWH=IWH,
                out_b=float(np.asarray(inputs["out_b"]).reshape(-1)[0]))

    # ---- vectorized edge -> slot assignment ----
    order = np.argsort(key, kind="stable")  # edges grouped by (core, blk, half)
    starts = np.zeros(NCORES * NBLK * 2 + 1, np.int64)
    starts[1:] = np.cumsum(cnts.reshape(-1))
    ks = key[order]
    rank = np.arange(E, dtype=np.int64) - starts[ks]
    blk_s = blk_of[order]
    slot = ecol[blk_s] + half[order] * n_lo[blk_s] + rank
    gslot = core_of[order] * EP + slot
    perm = np.full(NCORES * EP, -1, np.int64)
    perm[gslot] = order
    valid = perm >= 0
    pc = np.where(valid, perm, 0)

    # permuted edge features (bias folded in on device via per-partition add)
    eat = ea[pc]
    eat[~valid] = 0.0
    eat = eat.astype(BF).reshape(NCORES, EP, EFD)

    # per-slot one-hot row indices: 255 -> no match -> zero column/row
    arow_s = arow[pc]
    rexp_all = np.where(valid, arow_s, 255).astype(BF).reshape(NCORES, 1, EP)
    ragg_all = np.where(valid & (mask[pc] > 0), arow_s, 255).astype(np.float32)
    raggw_all = np.ascontiguousarray(
        ragg_all.reshape(NCORES, EP // 128, 128).transpose(0, 2, 1))

    # gather indices (src node ids) for lo/hi slot regions
    blkslot = np.repeat(np.arange(NBLK), nblk_e)
    islo = (np.arange(EP) - ecol[blkslot]) < n_lo[blkslot]
    gsrc = np.where(valid, src[pc], 0).reshape(NCORES, EP)
    ghi = np.where(valid, src[pc] - LOS, 0).reshape(NCORES, EP)
    lo_idx = gsrc[:, islo]
    hi_idx = ghi[:, ~islo]

    # shared weights
    eW1 = np.asarray(inputs["eW1"], np.float32)
    eW2 = np.asarray(inputs["eW2"], np.float32)
    nW1 = np.asarray(inputs["nW1"], np.float32)
    nW2 = np.asarray(inputs["nW2"], np.float32)

    def bfc(x):
        return np.ascontiguousarray(x, np.float32).astype(BF)

    atomW93 = np.zeros((AFD + 1, ND), np.float32)
    atomW93[:AFD] = np.asarray(inputs["atom_W"], np.float32)
    atomW93[AFD] = np.asarray(inputs["atom_b"], np.float32)

    we_dup = np.zeros((128, L * HID), np.float32)
    nw1b_dup = np.zeros((128, L * ND), np.float32)
    for l in range(L):
        we_dup[0:64, l * HID:(l + 1) * HID] = eW1[l, 0:ED]
        we_dup[64:128, l * HID:(l + 1) * HID] = eW1[l, 0:ED]
        nw1b_dup[0:64, l * ND:(l + 1) * ND] = nW1[l, ND:ND + ED]
        nw1b_dup[64:128, l * ND:(l + 1) * ND] = nW1[l, ND:ND + ED]
    # ws|wd packed per layer so node_tables does one matmul per block
    wswd = np.concatenate(
        [np.concatenate([eW1[l, ED:ED + ND], eW1[l, ED + ND:]], 1) for l in range(L)], 1)
    ew2_all = np.concatenate([eW2[l] for l in range(L)], 1)                 # [128, L*64]
    nw1a_all = np.concatenate([nW1[l, 0:ND] for l in range(L)], 1)          # [128, L*128]
    nw2_all = np.concatenate([nW2[l] for l in range(L)], 1)                 # [128, L*128]

    eb1 = np.asarray(inputs["eb1"], np.float32).T.copy()                    # [128, L]
    eb2p = np.zeros((128, L), np.float32)
    eb2p[0:64] = np.asarray(inputs["eb2"], np.float32).T
    eb2p[64:128] = eb2p[0:64]
    nb1 = np.asarray(inputs["nb1"], np.float32).T.copy()
    nb2 = np.asarray(inputs["nb2"], np.float32).T.copy()
    edgeb2 = np.zeros((128, 1), np.float32)
    edgeb2[0:64, 0] = np.asarray(inputs["edge_b"], np.float32)
    edgeb2[64:128, 0] = edgeb2[0:64, 0]

    i64d = np.zeros((128, 64), np.float32)
    i64d[0:64] = np.eye(64)
    i64d[64:128] = np.eye(64)

    # ---- pack all weights/constants into two buffers (per-tensor transfer
    # cost through the tunnel is ~tens of ms, so fewer tensors = faster) ----
    EPT = EP // 128
    WB = [("atomW", AFD + 1, ND), ("we_dup", 128, L * HID),
          ("nw1b_dup", 128, L * ND), ("wswd", 128, L * 256), ("ew2_all", HID, L * ED),
          ("nw1a_all", ND, L * HID), ("nw2_all", HID, L * ND), ("readW", ND, RD),
          ("outW", RD, 1), ("i64d", 128, 64), ("i128b", 128, 128)]
    WF = [("eb1", 128, L), ("eb2p", 128, L), ("nb1", 128, L), ("nb2", 128, L),
          ("readb", RD, 1), ("edgeb2", 128, 1), ("i128f", 128, 128),
          ("iotaF", 128, 128), ("iotap", 128, 1), ("invccnt", GC, NG),
          ("invcnt", 128, NBLK), ("cidw", 128, NBLK), ("raggw", 128, EPT)]

    def offsets(entries):
        offs, off = {}, 0
        for nm, r, c in entries:
            offs[nm] = (r, c, off)
            off += c
        return offs, off

    WBO, XB = offsets(WB)
    WFO, XF = offsets(WF)
    meta.update(WBO=WBO, WFO=WFO, XB=XB, XF=XF)

    vals_b = {
        "atomW": bfc(atomW93),
        "we_dup": bfc(we_dup), "nw1b_dup": bfc(nw1b_dup), "wswd": bfc(wswd),
        "ew2_all": bfc(ew2_all), "nw1a_all": bfc(nw1a_all), "nw2_all": bfc(nw2_all),
        "readW": bfc(np.asarray(inputs["read_W"])), "outW": bfc(np.asarray(inputs["out_W"])),
        "i64d": bfc(i64d), "i128b": bfc(np.eye(128)),
    }
    vals_f = {
        "eb1": eb1, "eb2p": eb2p, "nb1": nb1, "nb2": nb2, "edgeb2": edgeb2,
        "readb": np.asarray(inputs["read_b"], np.float32).reshape(RD, 1),
        "i128f": np.eye(128, dtype=np.float32),
        "iotaF": np.tile(np.arange(128, dtype=np.float32), (128, 1)),
        "iotap": np.arange(128, dtype=np.float32).reshape(128, 1),
        "invccnt": np.pad(invccnt, (0, NG * GC - NCRYS)).reshape(NG, GC).T.copy(),
    }
    wb = np.zeros((128, XB), BF)
    for nm, (r, c, off) in WBO.items():
        wb[0:r, off:off + c] = vals_b[nm]

    in_maps = []
    for k in range(NCORES):
        a0 = k * ASH
        inv_sb = np.ones((128, NBLK), np.float32)
        cidw = np.full((128, NBLK), -1.0, np.float32)
        for b in range(NBLK):
            na = min(128, ASH - 128 * b)
            inv_sb[0:na, b] = invcnt[a0 + 128 * b: a0 + 128 * b + na]
            cidw[0:na, b] = cb[a0 + 128 * b: a0 + 128 * b + na]
        afT = np.zeros((AFD + 1, ASH), np.float32)
        afT[:AFD] = af[a0:a0 + ASH].T
        afT[AFD] = 1.0
        wf = np.zeros((128, XF), np.float32)
        for nm, (r, c, off) in WFO.items():
            v = vals_f.get(nm)
            if nm == "invcnt":
                v = inv_sb
            elif nm == "cidw":
                v = cidw
            elif nm == "raggw":
                v = raggw_all[k]
            wf[0:r, off:off + c] = v

        m = {
            # edge features + edge_W quantized to fp8e4m3 (features are ~N(0,1);
            # ~3% element error stays far inside the accuracy budget)
            "eat": np.concatenate(
                [np.ascontiguousarray(eat[k].T),
                 np.asarray(inputs["edge_W"], np.float32)], 1).astype(F8E4),
            "rexp": rexp_all[k],
            "idxs": np.concatenate(
                [_wrap_idx(lo_idx[k]), _wrap_idx(hi_idx[k])], 1) if IWH
                else _wrap_idx(lo_idx[k]),
            "afT": afT.astype(BF), "wb": wb, "wf": wf,
        }
        in_maps.append(m)
    return meta, in_maps


def _build(meta, act=AFT.Silu, noop=False, no_gather=False, no_coll=False):
    cfg = meta["cfg"]
    N, M, AFD, EFD, NCRYS, L = (cfg[k] for k in ("N", "M", "AFD", "EFD", "NCRYS", "L"))
    ASH, NBLK, LOS, NG = meta["ASH"], meta["NBLK"], meta["LOS"], meta["NG"]
    EP, EPC, blocks = meta["EP"], meta["EPC"], meta["blocks"]
    IWL, IWH = meta["IWL"], meta["IWH"]
    EPT = EP // 128

    WBO, WFO, XB, XF = meta["WBO"], meta["WFO"], meta["XB"], meta["XF"]

    nc = bacc.Bacc("TRN2", target_bir_lowering=False, debug=False, num_devices=NCORES,
                   num_swdge_queues=4)

    def din(name, shape, dt):
        return nc.dram_tensor(name, shape, dt, kind="ExternalInput")

    eat_d = din("eat", [EFD, EP + ED], F8)       # fp8 edge features | edge_W
    rexp_d = din("rexp", [1, EP], BF16)
    idxs_d = din("idxs", [16, IWL + IWH], I16)
    afT = din("afT", [AFD + 1, ASH], BF16)
    wb_d = din("wb", [128, XB], BF16)
    wf_d = din("wf", [128, XF], F32)
    y = nc.dram_tensor("y", [1, NCRYS], F32, kind="ExternalOutput")

    if noop:
        with tile.TileContext(nc) as tc:
            with tc.tile_pool(name="sbz", bufs=1) as sbz:
                yz = sbz.tile([1, NCRYS], F32, tag="yz")
                nc.gpsimd.memset(yz[:], 0.0)
                nc.sync.dma_start(y[:], yz[:])
        nc.compile()
        return nc

    with tile.TileContext(nc) as tc:
        with (
            tc.tile_pool(name="persist", bufs=1) as pp,
            tc.tile_pool(name="dram", bufs=1, space="DRAM") as dp,
        ):
            nc.gpsimd.load_library(library_config.mlp)
            w = {}
            for nm, (r, c, off) in WBO.items():
                w[nm] = pp.tile([r, c], BF16, tag=nm, name=f"w_{nm}")
                nc.sync.dma_start(w[nm][:], wb_d[0:r, off:off + c])
            for nm, (r, c, off) in WFO.items():
                if nm == "raggw":
                    continue  # init-only; loaded into the init pool below
                w[nm] = pp.tile([r, c], F32, tag=nm, name=f"w_{nm}")
                nc.sync.dma_start(w[nm][:], wf_d[0:r, off:off + c])
            w["edgeW"] = pp.tile([EFD, ED], F8, tag="edgeW", name="w_edgeW")
            nc.sync.dma_start(w["edgeW"][:], eat_d[:, EP:EP + ED])
            invcnt_sb = w["invcnt"]
            stateT = pp.tile([128, EPC], BF16, tag="stateT")
            nodeT = pp.tile([128, ASH], F32, tag="nodeT")
            nodeTb = pp.tile([128, ASH], BF16, tag="nodeTb")
            adst = pp.tile([128, NBLK * 128], BF16, tag="adst")
            aggT = pp.tile([128, _cdiv(NBLK, 2) * 128], BF16, tag="aggT")
            ones1 = pp.tile([1, 128], BF16, tag="ones1")
            nc.vector.memset(ones1[:], 1.0)
            idxsb = pp.tile([128, IWL], I16, tag="idxsb")
            for r in range(8):
                nc.sync.dma_start(idxsb[16 * r:16 * r + 16, :], idxs_d[:, 0:IWL])
            if IWH:
                idxsbh = pp.tile([128, IWH], I16, tag="idxsbh")
                for r in range(8):
                    nc.sync.dma_start(idxsbh[16 * r:16 * r + 16, :], idxs_d[:, IWL:IWL + IWH])
            ssdev = dp.tile([128, 2 * EP], BF16)  # device-built one-hot scatter mats
            asrc_in = dp.tile([ASH, ND], BF16)
            asrc_fulls = [dp.tile([N, ND], BF16, addr_space="Shared", name=f"asrc_full{i}", tag=f"asrc_full{i}")
                          for i in range(L)]
            pool_in = dp.tile([NCRYS, ND], F32)
            pool_out = dp.tile([NCRYS, ND], F32, addr_space="Shared")

            def node_tables(lw, sbp, psp):
                """A_src shard -> bounce -> AllGather; A_dst blocks (layer lw)."""
                for t in range(NBLK):
                    na = min(128, ASH - 128 * t)
                    lhs = nodeTb[:, 128 * t:128 * t + na]
                    ps_s = psp.tile([128, 256], F32, tag="ps_s")
                    nc.tensor.matmul(ps_s[0:na, :], lhs, w["wswd"][:, lw * 256:(lw + 1) * 256],
                                     start=True, stop=True)
                    asb = sbp.tile([128, 128], BF16, tag="asb")
                    nc.vector.tensor_copy(asb[0:na, :], ps_s[0:na, 0:128])
                    nc.sync.dma_start(asrc_in[128 * t:128 * t + na, :], asb[0:na, :])
                    nc.vector.tensor_copy(adst[0:na, 128 * t:128 * t + 128][:, 0:128],
                                          ps_s[0:na, 128:256])
                if not no_coll:
                    nc.gpsimd.collective_compute(
                        "AllGather", mybir.AluOpType.bypass,
                        replica_groups=[list(range(NCORES))],
                        ins=[asrc_in[:].opt()], outs=[asrc_fulls[lw][:].opt()],
                    )
                else:
                    nc.sync.dma_start(asrc_fulls[lw][0:ASH, :], asrc_in[:])

            # ---- init: projections + device-side one-hot build + layer-0 tables ----
            with tc.tile_pool(name="sbi", bufs=3) as sbp, \
                 tc.tile_pool(name="psi", bufs=2, space="PSUM") as psp:
                rr, rc, roff = WFO["raggw"]
                raggw_sb = sbp.tile([128, EPT], F32, tag="raggw_sb", bufs=1)
                nc.sync.dma_start(raggw_sb[:], wf_d[0:rr, roff:roff + rc])
                for t in range(NBLK):
                    na = min(128, ASH - 128 * t)
                    aft = sbp.tile([AFD + 1, 128], BF16, tag="aft")
                    nc.sync.dma_start(aft[:, 0:na], afT[:, 128 * t:128 * t + na])
                    ps_n = psp.tile([128, 128], F32, tag="ps_n")
                    nc.tensor.matmul(ps_n[:, 0:na], w["atomW"][:], aft[:, 0:na],
                                     start=True, stop=True)
                    nc.vector.tensor_copy(nodeT[:, 128 * t:128 * t + na], ps_n[:, 0:na])
                    nc.vector.tensor_copy(nodeTb[:, 128 * t:128 * t + na], ps_n[:, 0:na])
                for blk in blocks:
                    hr = slice(64, 128) if blk["half"] else slice(0, 64)
                    for (sco, eco, bco, n) in blk["chunks"]:
                        eat = sbp.tile([EFD, 512], F8, tag="eat")
                        nc.sync.dma_start(eat[:, 0:n], eat_d[0:EFD, eco:eco + n])
                        ps_e = psp.tile([128, 512], F32, tag="ps_e")
                        nc.tensor.matmul(ps_e[hr, 0:n], w["edgeW"][:], eat[:, 0:n],
                                         start=True, stop=True)
                        nc.vector.tensor_scalar(stateT[hr, sco:sco + n], ps_e[hr, 0:n],
                                                w["edgeb2"][hr, 0:1], None, op0=ALU.add)
                        # expand per-slot row ids into one-hot scatter mats -> DRAM
                        rx = sbp.tile([1, 512], BF16, tag="rx")
                        nc.sync.dma_start(rx[0:1, 0:n], rexp_d[0:1, eco:eco + n])
                        ps_b = psp.tile([128, 512], F32, tag="ps_e")
                        nc.tensor.matmul(ps_b[:, 0:n], ones1[0:1, :], rx[0:1, 0:n],
                                         start=True, stop=True)
                        sst = sbp.tile([128, 1024], BF16, tag="sst")
                        nc.vector.tensor_scalar(sst[:, 0:n], ps_b[:, 0:n],
                                                w["iotap"][:, 0:1], None, op0=ALU.is_equal)
                        g0 = eco // 128
                        for j in range(n // 128):
                            nc.vector.tensor_scalar(sst[:, n + 128 * j:n + 128 * j + 128],
                                                    w["iotaF"][:],
                                                    raggw_sb[:, g0 + j:g0 + j + 1], None,
                                                    op0=ALU.is_equal)
                        nc.sync.dma_start(ssdev[:, 2 * eco:2 * eco + 2 * n], sst[:, 0:2 * n])
                node_tables(0, sbp, psp)

            # ---- layers ----
            for l in range(L):
                with tc.tile_pool(name=f"sbe{l}", bufs=3) as sbp, \
                     tc.tile_pool(name=f"pse{l}", bufs=2, space="PSUM") as psp, \
                     tc.tile_pool(name=f"psg{l}", bufs=2, space="PSUM") as psg:
                    for blk in blocks:
                        b = blk["b"]
                        hr = slice(64, 128) if blk["half"] else slice(0, 64)
                        ba = min(128, ASH - 128 * b)
                        asrc_full = asrc_fulls[l]
                        gt = sbp.tile([128, 1, blk["nblk_e"]], BF16, tag="gt", bufs=2)
                        if blk["n_lo"] and not no_gather:
                            io = sum(bb["n_lo"] for bb in blocks[:b]) // 16
                            nc.gpsimd.dma_gather(
                                gt[:, :, 0:blk["n_lo"]], asrc_full[0:LOS, :],
                                idxsb[:, io:io + blk["n_lo"] // 16],
                                blk["n_lo"], blk["n_lo"], ND, transpose=True,
                                queue_num=(2 * b) % 4)
                        if blk["n_hi"] and not no_gather:
                            io = sum(bb["n_hi"] for bb in blocks[:b]) // 16
                            nc.gpsimd.dma_gather(
                                gt[:, :, blk["n_lo"]:], asrc_full[LOS:N, :],
                                idxsbh[:, io:io + blk["n_hi"] // 16],
                                blk["n_hi"], blk["n_hi"], ND, transpose=True,
                                queue_num=(2 * b + 1) % 4)
                        ps_agg = psg.tile([128, 64], F32, tag="agg")
                        nchunk = len(blk["chunks"])
                        e0 = blk["chunks"][0][1]
                        ssb = sbp.tile([128, 2 * blk["nblk_e"]], BF16, tag="ssb", bufs=2)
                        nc.sync.dma_start(ssb[:, 0:2 * blk["nblk_e"]],
                                          ssdev[:, 2 * e0:2 * e0 + 2 * blk["nblk_e"]])
                        for ci, (sco, eco, bco, n) in enumerate(blk["chunks"]):
                            sst = ssb[:, 2 * (eco - e0):2 * (eco - e0) + 2 * n]
                            ps_h = psp.tile([128, 512], F32, tag="ph")
                            nc.tensor.matmul(ps_h[:, 0:n], adst[0:ba, 128 * b:128 * b + 128],
                                             sst[0:ba, 0:n], start=True, stop=False)  # S^T chunk
                            nc.tensor.matmul(ps_h[:, 0:n], w["we_dup"][hr, l * HID:(l + 1) * HID],
                                             stateT[hr, sco:sco + n], start=False,
                                             stop=True)
                            ht = sbp.tile([128, 512], BF16, tag="ht")
                            if not no_gather:
                                # gathered A_src + eb1 fused on DVE instead of a
                                # third accumulating matmul on PE
                                hpre = sbp.tile([128, 512], BF16, tag="hpre")
                                nc.vector.scalar_tensor_tensor(
                                    hpre[:, 0:n], ps_h[:, 0:n], w["eb1"][:, l:l + 1],
                                    gt[:, 0, bco:bco + n], op0=ALU.add, op1=ALU.add)
                                nc.scalar.activation(ht[:, 0:n], hpre[:, 0:n], act)
                            else:
                                nc.scalar.activation(ht[:, 0:n], ps_h[:, 0:n], act,
                                                     bias=w["eb1"][:, l:l + 1])
                            ps_dd = psp.tile([128, 512], F32, tag="pd")
                            nc.tensor.matmul(ps_dd[hr, 0:n], w["ew2_all"][:, l * ED:(l + 1) * ED],
                                             ht[:, 0:n], start=True, stop=True)
                            # state += mlp_out + eb2 (residual on DVE, not PE)
                            nc.vector.scalar_tensor_tensor(
                                stateT[hr, sco:sco + n], ps_dd[hr, 0:n],
                                w["eb2p"][hr, l:l + 1], stateT[hr, sco:sco + n],
                                op0=ALU.add, op1=ALU.add)
                            ps_t = psp.tile([128, 256], BF16, tag="pt", bufs=1)
                            for j in range(n // 128):
                                nc.tensor.transpose(
                                    ps_t[:, 64 * j:64 * j + 64],
                                    stateT[hr, sco + 128 * j:sco + 128 * j + 128],
                                    w["i64d"][hr, :])
                            nn = sbp.tile([128, 256], BF16, tag="nn")
                            nc.vector.tensor_copy(nn[:, 0:64 * (n // 128)], ps_t[:, 0:64 * (n // 128)])
                            for j in range(n // 128):
                                nc.tensor.matmul(
                                    ps_agg[:],
                                    sst[:, n + 128 * j:n + 128 * j + 128],
                                    nn[:, 64 * j:64 * j + 64],
                                    start=(ci == 0 and j == 0),
                                    stop=(ci == nchunk - 1 and j == n // 128 - 1))
                        agnb = sbp.tile([128, 64], BF16, tag="agnb")
                        nc.scalar.activation(agnb[:], ps_agg[:], AFT.Identity,
                                             scale=invcnt_sb[:, b:b + 1])
                        ps_at = psp.tile([128, 128], BF16, tag="pat", bufs=1)
                        hr2 = slice(64, 128) if b % 2 else slice(0, 64)
                        nc.tensor.transpose(ps_at[hr2, :], agnb[:], w["i128b"][:])
                        nc.vector.tensor_copy(aggT[hr2, (b // 2) * 128:(b // 2) * 128 + 128],
                                              ps_at[hr2, :])
                # node MLP + next-layer tables
                with tc.tile_pool(name=f"sbn{l}", bufs=3) as sbp, \
                     tc.tile_pool(name=f"psn{l}", bufs=2, space="PSUM") as psp:
                    for t in range(NBLK):
                        na = min(128, ASH - 128 * t)
                        hr2 = slice(64, 128) if t % 2 else slice(0, 64)
                        ps_hn = psp.tile([128, 128], F32, tag="hn")
                        nc.tensor.matmul(ps_hn[:, 0:na],
                                         w["nw1a_all"][:, l * HID:(l + 1) * HID],
                                         nodeTb[:, 128 * t:128 * t + na],
                                         start=True, stop=False)
                        nc.tensor.matmul(ps_hn[:, 0:na],
                                         w["nw1b_dup"][hr2, l * HID:(l + 1) * HID],
                                         aggT[hr2, (t // 2) * 128:(t // 2) * 128 + na],
                                         start=False, stop=True)
                        hn = sbp.tile([128, 128], BF16, tag="hn_s")
                        nc.scalar.activation(hn[:, 0:na], ps_hn[:, 0:na], act,
                                             bias=w["nb1"][:, l:l + 1])
                        ps_nd = psp.tile([128, 128], F32, tag="ndl")
                        nc.tensor.matmul(ps_nd[:, 0:na],
                                         w["nw2_all"][:, l * ND:(l + 1) * ND],
                                         hn[:, 0:na], start=True, stop=True)
                        nc.vector.scalar_tensor_tensor(
                            nodeT[:, 128 * t:128 * t + na], ps_nd[:, 0:na],
                            w["nb2"][:, l:l + 1], nodeT[:, 128 * t:128 * t + na],
                            op0=ALU.add, op1=ALU.add)
                        nc.vector.tensor_copy(nodeTb[:, 128 * t:128 * t + na],
                                              nodeT[:, 128 * t:128 * t + na])
                    if l < L - 1:
                        node_tables(l + 1, sbp, psp)

            # ---- pooling ----
            with tc.tile_pool(name="sbt", bufs=3) as sbt, \
                 tc.tile_pool(name="pst", bufs=2, space="PSUM") as pst:
                nnat_all = pp.tile([128, NBLK * 128], BF16, tag="nnat_all")
                for t in range(NBLK):
                    na = min(128, ASH - 128 * t)
                    ps_tr = pst.tile([128, 128], F32, tag="ptr")
                    nc.tensor.transpose(ps_tr[0:na, :], nodeT[:, 128 * t:128 * t + na],
                                        w["i128f"][:])
                    nc.vector.tensor_copy(nnat_all[0:na, 128 * t:128 * t + 128][:, 0:128],
                                          ps_tr[0:na, :])
            with tc.tile_pool(name="sbp", bufs=3) as sbp, \
                 tc.tile_pool(name="psp", bufs=1, space="PSUM") as psp:
                cidw_sb = w["cidw"]
                iota_c = sbp.tile([128, NG * GC], F32, tag="iotac", bufs=1)
                for q in range(_cdiv(NG * GC, 128)):
                    qn = min(128, NG * GC - 128 * q)
                    nc.vector.tensor_scalar(iota_c[:, 128 * q:128 * q + qn],
                                            w["iotaF"][:, 0:qn], float(128 * q), None,
                                            op0=ALU.add)
                pools = [psp.tile([128, 128], F32, tag=f"pool{g}", name=f"pool{g}") for g in range(NG)]
                for t in range(NBLK):
                    na = min(128, ASH - 128 * t)
                    pmt = sbp.tile([128, NG * GC], BF16, tag="pmt")
                    nc.vector.tensor_scalar(pmt[:], iota_c[:], cidw_sb[:, t:t + 1], None,
                                            op0=ALU.is_equal)
                    for g in range(NG):
                        gc = min(GC, NCRYS - g * GC)
                        nc.tensor.matmul(pools[g][0:gc, :], pmt[0:na, g * GC:g * GC + gc],
                                         nnat_all[0:na, 128 * t:128 * t + 128][:, 0:128],
                                         start=(t == 0), stop=(t == NBLK - 1))
                for g in range(NG):
                    gc = min(GC, NCRYS - g * GC)
                    pev = sbp.tile([128, 128], F32, tag="pev")
                    nc.vector.tensor_copy(pev[0:gc, :], pools[g][0:gc, :])
                    nc.sync.dma_start(pool_in[g * GC:g * GC + gc, :], pev[0:gc, :])
                if not no_coll:
                    nc.gpsimd.collective_compute(
                        "AllReduce", mybir.AluOpType.add,
                        replica_groups=[list(range(NCORES))],
                        ins=[pool_in[:].opt()], outs=[pool_out[:].opt()],
                    )
                else:
                    nc.sync.dma_start(pool_out[:], pool_in[:])

            # ---- readout (replicated) ----
            with tc.tile_pool(name="sbr", bufs=2) as sbp, \
                 tc.tile_pool(name="psr", bufs=2, space="PSUM") as psp:
                for g in range(NG):
                    gc = min(GC, NCRYS - g * GC)
                    pg = sbp.tile([128, 128], F32, tag="pg")
                    nc.sync.dma_start(pg[0:gc, :], pool_out[g * GC:g * GC + gc, :])
                    mean = sbp.tile([128, 128], BF16, tag="mean")
                    nc.scalar.activation(mean[0:gc, :], pg[0:gc, :], AFT.Identity,
                                         scale=w["invccnt"][0:gc, g:g + 1])
                    ps_mt = psp.tile([128, 128], BF16, tag="pmt2")
                    nc.tensor.transpose(ps_mt[:, 0:gc], mean[0:gc, :], w["i128b"][0:gc, 0:gc])
                    meanT = sbp.tile([128, 128], BF16, tag="meanT")
                    nc.vector.tensor_copy(meanT[:, 0:gc], ps_mt[:, 0:gc])
                    ps_hr = psp.tile([128, 128], F32, tag="phr")
                    nc.tensor.matmul(ps_hr[:, 0:gc], w["readW"][:], meanT[:, 0:gc],
                                     start=True, stop=True)
                    hrT = sbp.tile([128, 128], BF16, tag="hrT")
                    nc.scalar.activation(hrT[:, 0:gc], ps_hr[:, 0:gc], act,
                                         bias=w["readb"][:])
                    ps_y = psp.tile([128, 128], F32, tag="py")
                    nc.tensor.matmul(ps_y[0:1, 0:gc], w["outW"][:], hrT[:, 0:gc],
                                     start=True, stop=True)
                    ysb = sbp.tile([1, 128], F32, tag="ysb")
                    nc.scalar.activation(ysb[0:1, 0:gc], ps_y[0:1, 0:gc], AFT.Copy,
                                         bias=meta["out_b"])
                    nc.sync.dma_start(y[0:1, g * GC:g * GC + gc], ysb[0:1, 0:gc])

    nc.compile()
    return nc


def run_cores(meta, in_maps, act=AFT.Silu, sim=False):
    nc = _build(meta, act=act)
    if sim:
        from concourse.bass_interp import MultiCoreSim
        s = MultiCoreSim(nc, NCORES, trace=False)
        for k in range(NCORES):
            for nm, arr in in_maps[k].items():
                s.cores[k].tensor(nm)[:] = arr
        s.simulate(check_with_hw=False)
        return [{"y": np.array(s.cores[k].tensor("y"))} for k in range(NCORES)], None
    from concourse import bass_utils
    res = bass_utils.run_bass_kernel_spmd(nc, in_maps, core_ids=list(range(NCORES)))
    return res.results, res


def kernel(**inputs):
    cfg = dict(FULL_CFG)
    n, m = np.asarray(inputs["nbr_fea_idx"]).shape
    cfg["N"], cfg["M"] = int(n), int(m)
    cfg["AFD"] = int(np.asarray(inputs["atom_fea"]).shape[1])
    cfg["EFD"] = int(np.asarray(inputs["nbr_fea"]).shape[2])
    cfg["NCRYS"] = int(inputs["num_crystals"])
    cfg["L"] = int(np.asarray(inputs["eW1"]).shape[0])
    meta, in_maps = _prep(inputs, cfg)
    results, _ = run_cores(meta, in_maps)
    return np.asarray(results[0]["y"], np.float32).reshape(cfg["NCRYS"], 1)



# revision 7
# speedup vs baseline: 23.6937x; 23.6937x over previous
"""CrystalGraphALIGNN Trainium2 kernel (8 NeuronCores, SPMD).

Strategy: dst-shard edges across cores (atom v owned by core v // (N/8); edge
(i,j) owned by the core of its dst). Per core, edges are sorted by dst and
grouped into 128-atom blocks so that:
  - the dst-side expansion A_dst[dst(e)] is a block-local one-hot matmul,
  - the scatter-mean aggregation is a one-hot matmul into PSUM,
  - only the src side needs a true random gather: per-edge rows of
    A_src = node @ W_src, fetched with dma_gather (transposed, bf16) from a
    DRAM table that is refreshed once per layer via AllGather.
The block-local one-hot matrices are never shipped from the host: the host
sends one bf16/f32 row index per edge slot and the kernel expands them into
a DRAM scratch at init with a rank-1 broadcast matmul + is_equal compare.
Node states and the node MLP stay fully shard-local; crystal pooling is a
one-hot matmul (also built on device from per-atom crystal ids) + a single
AllReduce, readout replicated on every core.

_build(repeat=R) emits the whole computation R times in one program (weights
loaded once); timing builds with different R and taking the delta cancels
the per-call host/tunnel dispatch overhead, which is how test.py measures
the hardware execution time without NTFF profiling.
"""

import numpy as np
import ml_dtypes

import concourse.bass as bass
import concourse.bacc as bacc
import concourse.mybir as mybir
import concourse.tile as tile
from concourse import library_config

F32 = mybir.dt.float32
BF16 = mybir.dt.bfloat16
F8 = mybir.dt.float8e4
I16 = mybir.dt.int16
AFT = mybir.ActivationFunctionType
ALU = mybir.AluOpType
BF = ml_dtypes.bfloat16
F8E4 = ml_dtypes.float8_e4m3

NCORES = 8
ED, ND, HID, RD = 64, 128, 128, 128
EDGE_THRESH = 1e-6
GC = 125  # crystals per pooling group

FULL_CFG = dict(N=50000, M=12, AFD=92, EFD=41, NCRYS=1000, L=4)


def _cdiv(a, b):
    return (a + b - 1) // b


def _wrap_idx(flat):
    """int16 flat idx [n] -> [16, n/16] wrapped layout (replicated on device)."""
    n = flat.shape[-1]
    assert n % 16 == 0
    return np.ascontiguousarray(flat.reshape(n // 16, 16).T.astype(np.int16))


def _prep(inputs, cfg):
    N, M, AFD, EFD, NCRYS, L = (cfg[k] for k in ("N", "M", "AFD", "EFD", "NCRYS", "L"))
    ASH = N // NCORES
    NBLK = _cdiv(ASH, 128)
    LOS = min(25000, N)  # src index split for int16 gather indices
    NG = _cdiv(NCRYS, GC)

    af = np.asarray(inputs["atom_fea"], np.float32)
    nf = np.asarray(inputs["nbr_fea"], np.float32)
    nidx = np.asarray(inputs["nbr_fea_idx"]).astype(np.int64)
    cb = np.asarray(inputs["crystal_batch"]).astype(np.int64)

    E = N * M
    dst = np.clip(nidx.reshape(-1), 0, N - 1)
    src = np.repeat(np.arange(N, dtype=np.int64), M)
    ea = nf.reshape(E, EFD)
    mask = (np.abs(ea).sum(1) > EDGE_THRESH).astype(np.float32)

    cnt = np.bincount(dst, weights=mask, minlength=N)
    invcnt = (1.0 / np.maximum(cnt, 1.0)).astype(np.float32)
    ccnt = np.bincount(cb, minlength=NCRYS).astype(np.float32)
    invccnt = (1.0 / np.maximum(ccnt, 1.0)).astype(np.float32)

    core_of = dst // ASH
    dloc = dst - core_of * ASH
    blk_of = dloc // 128
    arow = dloc - blk_of * 128
    half = (src >= LOS).astype(np.int64)

    # per-(core, block, half) edge counts -> shared tile geometry
    key = (core_of * NBLK + blk_of) * 2 + half
    cnts = np.bincount(key, minlength=NCORES * NBLK * 2).reshape(NCORES, NBLK, 2)
    T_lo = np.maximum((cnts[:, :, 0].max(0) + 127) // 128, 1)
    T_hi = (cnts[:, :, 1].max(0) + 127) // 128
    odd = ((T_lo + T_hi) % 2).astype(np.int64)
    if N > LOS:
        T_hi = T_hi + odd
    else:
        T_lo = T_lo + odd
    n_lo = T_lo * 128
    n_hi = T_hi * 128
    nblk_e = n_lo + n_hi
    ecol = np.zeros(NBLK + 1, np.int64)
    ecol[1:] = np.cumsum(nblk_e)
    EP = int(ecol[NBLK])

    # geometry: edge-col space (block-major), state-col space (per half),
    # chunk list entries: (state_col, edge_col, blk_edge_col, n)
    BHALF = NBLK // 2
    scol = np.zeros(NBLK, np.int64)
    acc = [0, 0]
    blocks = []
    for b in range(NBLK):
        hb = 0 if b < BHALF else 1
        scol[b] = acc[hb]
        acc[hb] += int(nblk_e[b])
        tiles = int(T_lo[b] + T_hi[b])
        chunks = []
        off = 0
        while tiles > 0:
            t = 4 if tiles >= 4 else tiles
            chunks.append((int(scol[b] + off), int(ecol[b] + off), off, t * 128))
            off += t * 128
            tiles -= t
        blocks.append(dict(b=b, half=hb, nblk_e=int(nblk_e[b]), chunks=chunks,
                           n_lo=int(n_lo[b]), n_hi=int(n_hi[b])))
    EPC = max(acc)
    IWL = int(T_lo.sum()) * 8
    IWH = int(T_hi.sum()) * 8

    meta = dict(cfg=cfg, ASH=ASH, NBLK=NBLK, LOS=LOS, NG=NG, EP=EP, EPC=EPC,
                BHALF=BHALF, blocks=blocks, IWL=IWL, IWH=IWH,
                out_b=float(np.asarray(inputs["out_b"]).reshape(-1)[0]))

    # ---- vectorized edge -> slot assignment ----
    order = np.argsort(key, kind="stable")  # edges grouped by (core, blk, half)
    starts = np.zeros(NCORES * NBLK * 2 + 1, np.int64)
    starts[1:] = np.cumsum(cnts.reshape(-1))
    ks = key[order]
    rank = np.arange(E, dtype=np.int64) - starts[ks]
    blk_s = blk_of[order]
    slot = ecol[blk_s] + half[order] * n_lo[blk_s] + rank
    gslot = core_of[order] * EP + slot
    perm = np.full(NCORES * EP, -1, np.int64)
    perm[gslot] = order
    valid = perm >= 0
    pc = np.where(valid, perm, 0)

    # permuted edge features (bias folded in on device via per-partition add)
    eat = ea[pc]
    eat[~valid] = 0.0
    eat = eat.astype(BF).reshape(NCORES, EP, EFD)

    # per-slot one-hot row indices: 255 -> no match -> zero column/row
    arow_s = arow[pc]
    rexp_all = np.where(valid, arow_s, 255).astype(BF).reshape(NCORES, 1, EP)
    ragg_all = np.where(valid & (mask[pc] > 0), arow_s, 255).astype(np.float32)
    raggw_all = np.ascontiguousarray(
        ragg_all.reshape(NCORES, EP // 128, 128).transpose(0, 2, 1))

    # gather indices (src node ids) for lo/hi slot regions
    blkslot = np.repeat(np.arange(NBLK), nblk_e)
    islo = (np.arange(EP) - ecol[blkslot]) < n_lo[blkslot]
    gsrc = np.where(valid, src[pc], 0).reshape(NCORES, EP)
    ghi = np.where(valid, src[pc] - LOS, 0).reshape(NCORES, EP)
    lo_idx = gsrc[:, islo]
    hi_idx = ghi[:, ~islo]

    # shared weights
    eW1 = np.asarray(inputs["eW1"], np.float32)
    eW2 = np.asarray(inputs["eW2"], np.float32)
    nW1 = np.asarray(inputs["nW1"], np.float32)
    nW2 = np.asarray(inputs["nW2"], np.float32)

    def bfc(x):
        return np.ascontiguousarray(x, np.float32).astype(BF)

    atomW93 = np.zeros((AFD + 1, ND), np.float32)
    atomW93[:AFD] = np.asarray(inputs["atom_W"], np.float32)
    atomW93[AFD] = np.asarray(inputs["atom_b"], np.float32)

    we_dup = np.zeros((128, L * HID), np.float32)
    nw1b_dup = np.zeros((128, L * ND), np.float32)
    for l in range(L):
        we_dup[0:64, l * HID:(l + 1) * HID] = eW1[l, 0:ED]
        we_dup[64:128, l * HID:(l + 1) * HID] = eW1[l, 0:ED]
        nw1b_dup[0:64, l * ND:(l + 1) * ND] = nW1[l, ND:ND + ED]
        nw1b_dup[64:128, l * ND:(l + 1) * ND] = nW1[l, ND:ND + ED]
    # ws|wd packed per layer so node_tables does one matmul per block
    wswd = np.concatenate(
        [np.concatenate([eW1[l, ED:ED + ND], eW1[l, ED + ND:]], 1) for l in range(L)], 1)
    ew2_all = np.concatenate([eW2[l] for l in range(L)], 1)                 # [128, L*64]
    nw1a_all = np.concatenate([nW1[l, 0:ND] for l in range(L)], 1)          # [128, L*128]
    nw2_all = np.concatenate([nW2[l] for l in range(L)], 1)                 # [128, L*128]

    eb1 = np.asarray(inputs["eb1"], np.float32).T.copy()                    # [128, L]
    eb2p = np.zeros((128, L), np.float32)
    eb2p[0:64] = np.asarray(inputs["eb2"], np.float32).T
    eb2p[64:128] = eb2p[0:64]
    nb1 = np.asarray(inputs["nb1"], np.float32).T.copy()
    nb2 = np.asarray(inputs["nb2"], np.float32).T.copy()
    edgeb2 = np.zeros((128, 1), np.float32)
    edgeb2[0:64, 0] = np.asarray(inputs["edge_b"], np.float32)
    edgeb2[64:128, 0] = edgeb2[0:64, 0]

    i64d = np.zeros((128, 64), np.float32)
    i64d[0:64] = np.eye(64)
    i64d[64:128] = np.eye(64)

    # ---- pack all weights/constants into two buffers (per-tensor transfer
    # cost through the tunnel is ~tens of ms, so fewer tensors = faster) ----
    EPT = EP // 128
    WB = [("atomW", AFD + 1, ND), ("we_dup", 128, L * HID),
          ("nw1b_dup", 128, L * ND), ("wswd", 128, L * 256), ("ew2_all", HID, L * ED),
          ("nw1a_all", ND, L * HID), ("nw2_all", HID, L * ND), ("readW", ND, RD),
          ("outW", RD, 1), ("i64d", 128, 64), ("i128b", 128, 128)]
    WF = [("eb1", 128, L), ("eb2p", 128, L), ("nb1", 128, L), ("nb2", 128, L),
          ("readb", RD, 1), ("edgeb2", 128, 1), ("i128f", 128, 128),
          ("iotaF", 128, 128), ("iotap", 128, 1), ("invccnt", GC, NG),
          ("invcnt", 128, NBLK), ("cidw", 128, NBLK), ("raggw", 128, EPT)]

    def offsets(entries):
        offs, off = {}, 0
        for nm, r, c in entries:
            offs[nm] = (r, c, off)
            off += c
        return offs, off

    WBO, XB = offsets(WB)
    WFO, XF = offsets(WF)
    meta.update(WBO=WBO, WFO=WFO, XB=XB, XF=XF)

    vals_b = {
        "atomW": bfc(atomW93),
        "we_dup": bfc(we_dup), "nw1b_dup": bfc(nw1b_dup), "wswd": bfc(wswd),
        "ew2_all": bfc(ew2_all), "nw1a_all": bfc(nw1a_all), "nw2_all": bfc(nw2_all),
        "readW": bfc(np.asarray(inputs["read_W"])), "outW": bfc(np.asarray(inputs["out_W"])),
        "i64d": bfc(i64d), "i128b": bfc(np.eye(128)),
    }
    vals_f = {
        "eb1": eb1, "eb2p": eb2p, "nb1": nb1, "nb2": nb2, "edgeb2": edgeb2,
        "readb": np.asarray(inputs["read_b"], np.float32).reshape(RD, 1),
        "i128f": np.eye(128, dtype=np.float32),
        "iotaF": np.tile(np.arange(128, dtype=np.float32), (128, 1)),
        "iotap": np.arange(128, dtype=np.float32).reshape(128, 1),
        "invccnt": np.pad(invccnt, (0, NG * GC - NCRYS)).reshape(NG, GC).T.copy(),
    }
    wb = np.zeros((128, XB), BF)
    for nm, (r, c, off) in WBO.items():
        wb[0:r, off:off + c] = vals_b[nm]

    in_maps = []
    for k in range(NCORES):
        a0 = k * ASH
        inv_sb = np.ones((128, NBLK), np.float32)
        cidw = np.full((128, NBLK), -1.0, np.float32)
        for b in range(NBLK):
            na = min(128, ASH - 128 * b)
            inv_sb[0:na, b] = invcnt[a0 + 128 * b: a0 + 128 * b + na]
            cidw[0:na, b] = cb[a0 + 128 * b: a0 + 128 * b + na]
        afT = np.zeros((AFD + 1, ASH), np.float32)
        afT[:AFD] = af[a0:a0 + ASH].T
        afT[AFD] = 1.0
        wf = np.zeros((128, XF), np.float32)
        for nm, (r, c, off) in WFO.items():
            v = vals_f.get(nm)
            if nm == "invcnt":
                v = inv_sb
            elif nm == "cidw":
                v = cidw
            elif nm == "raggw":
                v = raggw_all[k]
            wf[0:r, off:off + c] = v

        m = {
            # edge features + edge_W quantized to fp8e4m3 (features are ~N(0,1);
            # ~3% element error stays far inside the accuracy budget)
            "eat": np.concatenate(
                [np.ascontiguousarray(eat[k].T),
                 np.asarray(inputs["edge_W"], np.float32)], 1).astype(F8E4),
            "rexp": rexp_all[k],
            "idxs": np.concatenate(
                [_wrap_idx(lo_idx[k]), _wrap_idx(hi_idx[k])], 1) if IWH
                else _wrap_idx(lo_idx[k]),
            "afT": afT.astype(BF), "wb": wb, "wf": wf,
        }
        in_maps.append(m)
    return meta, in_maps


def _build(meta, act=AFT.Silu, noop=False, no_gather=False, no_coll=False,
           repeat=1):
    cfg = meta["cfg"]
    N, M, AFD, EFD, NCRYS, L = (cfg[k] for k in ("N", "M", "AFD", "EFD", "NCRYS", "L"))
    ASH, NBLK, LOS, NG = meta["ASH"], meta["NBLK"], meta["LOS"], meta["NG"]
    EP, EPC, blocks = meta["EP"], meta["EPC"], meta["blocks"]
    IWL, IWH = meta["IWL"], meta["IWH"]
    EPT = EP // 128

    WBO, WFO, XB, XF = meta["WBO"], meta["WFO"], meta["XB"], meta["XF"]

    nc = bacc.Bacc("TRN2", target_bir_lowering=False, debug=False, num_devices=NCORES,
                   num_swdge_queues=4)

    def din(name, shape, dt):
        return nc.dram_tensor(name, shape, dt, kind="ExternalInput")

    eat_d = din("eat", [EFD, EP + ED], F8)       # fp8 edge features | edge_W
    rexp_d = din("rexp", [1, EP], BF16)
    idxs_d = din("idxs", [16, IWL + IWH], I16)
    afT = din("afT", [AFD + 1, ASH], BF16)
    wb_d = din("wb", [128, XB], BF16)
    wf_d = din("wf", [128, XF], F32)
    y = nc.dram_tensor("y", [1, NCRYS], F32, kind="ExternalOutput")

    if noop:
        with tile.TileContext(nc) as tc:
            with tc.tile_pool(name="sbz", bufs=1) as sbz:
                yz = sbz.tile([1, NCRYS], F32, tag="yz")
                nc.gpsimd.memset(yz[:], 0.0)
                nc.sync.dma_start(y[:], yz[:])
        nc.compile()
        return nc

    with tile.TileContext(nc) as tc:
        with (
            tc.tile_pool(name="persist", bufs=1) as pp,
            tc.tile_pool(name="dram", bufs=1, space="DRAM") as dp,
        ):
            nc.gpsimd.load_library(library_config.mlp)
            w = {}
            for nm, (r, c, off) in WBO.items():
                w[nm] = pp.tile([r, c], BF16, tag=nm, name=f"w_{nm}")
                nc.sync.dma_start(w[nm][:], wb_d[0:r, off:off + c])
            for nm, (r, c, off) in WFO.items():
                if nm == "raggw":
                    continue  # init-only; loaded into the init pool below
                w[nm] = pp.tile([r, c], F32, tag=nm, name=f"w_{nm}")
                nc.sync.dma_start(w[nm][:], wf_d[0:r, off:off + c])
            w["edgeW"] = pp.tile([EFD, ED], F8, tag="edgeW", name="w_edgeW")
            nc.sync.dma_start(w["edgeW"][:], eat_d[:, EP:EP + ED])
            invcnt_sb = w["invcnt"]
            stateT = pp.tile([128, EPC], BF16, tag="stateT")
            nodeT = pp.tile([128, ASH], F32, tag="nodeT")
            nodeTb = pp.tile([128, ASH], BF16, tag="nodeTb")
            adst = pp.tile([128, NBLK * 128], BF16, tag="adst")
            aggT = pp.tile([128, _cdiv(NBLK, 2) * 128], BF16, tag="aggT")
            nnat_all = pp.tile([128, NBLK * 128], BF16, tag="nnat_all")
            ones1 = pp.tile([1, 128], BF16, tag="ones1")
            nc.vector.memset(ones1[:], 1.0)
            idxsb = pp.tile([128, IWL], I16, tag="idxsb")
            for r in range(8):
                nc.sync.dma_start(idxsb[16 * r:16 * r + 16, :], idxs_d[:, 0:IWL])
            if IWH:
                idxsbh = pp.tile([128, IWH], I16, tag="idxsbh")
                for r in range(8):
                    nc.sync.dma_start(idxsbh[16 * r:16 * r + 16, :], idxs_d[:, IWL:IWL + IWH])
            ssdev = dp.tile([128, 2 * EP], BF16)  # device-built one-hot scatter mats

            def node_tables(lw, sbp, psp, asrc_in, asrc_fulls):
                """A_src shard -> bounce -> AllGather; A_dst blocks (layer lw)."""
                for t in range(NBLK):
                    na = min(128, ASH - 128 * t)
                    lhs = nodeTb[:, 128 * t:128 * t + na]
                    ps_s = psp.tile([128, 256], F32, tag="ps_s")
                    nc.tensor.matmul(ps_s[0:na, :], lhs, w["wswd"][:, lw * 256:(lw + 1) * 256],
                                     start=True, stop=True)
                    asb = sbp.tile([128, 128], BF16, tag="asb")
                    nc.vector.tensor_copy(asb[0:na, :], ps_s[0:na, 0:128])
                    nc.sync.dma_start(asrc_in[128 * t:128 * t + na, :], asb[0:na, :])
                    nc.vector.tensor_copy(adst[0:na, 128 * t:128 * t + 128][:, 0:128],
                                          ps_s[0:na, 128:256])
                if not no_coll:
                    nc.gpsimd.collective_compute(
                        "AllGather", mybir.AluOpType.bypass,
                        replica_groups=[list(range(NCORES))],
                        ins=[asrc_in[:].opt()], outs=[asrc_fulls[lw][:].opt()],
                    )
                else:
                    nc.sync.dma_start(asrc_fulls[lw][0:ASH, :], asrc_in[:])

            def emit_once(pfx):
                # collective buffers are per-rep: Shared DRAM outputs may only
                # have a single writer instruction
                asrc_in = dp.tile([ASH, ND], BF16, tag=f"{pfx}asrc_in")
                asrc_fulls = [dp.tile([N, ND], BF16, addr_space="Shared",
                                      name=f"{pfx}asrc_full{i}", tag=f"{pfx}asrc_full{i}")
                              for i in range(L)]
                pool_in = dp.tile([NCRYS, ND], F32, tag=f"{pfx}pool_in")
                pool_out = dp.tile([NCRYS, ND], F32, addr_space="Shared",
                                   tag=f"{pfx}pool_out")
                # ---- init: projections + device-side one-hot build + layer-0 tables ----
                with tc.tile_pool(name=pfx + "sbi", bufs=3) as sbp, \
                     tc.tile_pool(name=pfx + "psi", bufs=2, space="PSUM") as psp:
                    rr, rc, roff = WFO["raggw"]
                    raggw_sb = sbp.tile([128, EPT], F32, tag="raggw_sb", bufs=1)
                    nc.sync.dma_start(raggw_sb[:], wf_d[0:rr, roff:roff + rc])
                    for t in range(NBLK):
                        na = min(128, ASH - 128 * t)
                        aft = sbp.tile([AFD + 1, 128], BF16, tag="aft")
                        nc.sync.dma_start(aft[:, 0:na], afT[:, 128 * t:128 * t + na])
                        ps_n = psp.tile([128, 128], F32, tag="ps_n")
                        nc.tensor.matmul(ps_n[:, 0:na], w["atomW"][:], aft[:, 0:na],
                                         start=True, stop=True)
                        nc.vector.tensor_copy(nodeT[:, 128 * t:128 * t + na], ps_n[:, 0:na])
                        nc.vector.tensor_copy(nodeTb[:, 128 * t:128 * t + na], ps_n[:, 0:na])
                    for blk in blocks:
                        hr = slice(64, 128) if blk["half"] else slice(0, 64)
                        for (sco, eco, bco, n) in blk["chunks"]:
                            eat = sbp.tile([EFD, 512], F8, tag="eat")
                            nc.sync.dma_start(eat[:, 0:n], eat_d[0:EFD, eco:eco + n])
                            ps_e = psp.tile([128, 512], F32, tag="ps_e")
                            nc.tensor.matmul(ps_e[hr, 0:n], w["edgeW"][:], eat[:, 0:n],
                                             start=True, stop=True)
                            nc.vector.tensor_scalar(stateT[hr, sco:sco + n], ps_e[hr, 0:n],
                                                    w["edgeb2"][hr, 0:1], None, op0=ALU.add)
                            # expand per-slot row ids into one-hot scatter mats -> DRAM
                            rx = sbp.tile([1, 512], BF16, tag="rx")
                            nc.sync.dma_start(rx[0:1, 0:n], rexp_d[0:1, eco:eco + n])
                            ps_b = psp.tile([128, 512], F32, tag="ps_e")
                            nc.tensor.matmul(ps_b[:, 0:n], ones1[0:1, :], rx[0:1, 0:n],
                                             start=True, stop=True)
                            sst = sbp.tile([128, 1024], BF16, tag="sst")
                            nc.vector.tensor_scalar(sst[:, 0:n], ps_b[:, 0:n],
                                                    w["iotap"][:, 0:1], None, op0=ALU.is_equal)
                            g0 = eco // 128
                            for j in range(n // 128):
                                nc.vector.tensor_scalar(sst[:, n + 128 * j:n + 128 * j + 128],
                                                        w["iotaF"][:],
                                                        raggw_sb[:, g0 + j:g0 + j + 1], None,
                                                        op0=ALU.is_equal)
                            nc.sync.dma_start(ssdev[:, 2 * eco:2 * eco + 2 * n], sst[:, 0:2 * n])
                    node_tables(0, sbp, psp, asrc_in, asrc_fulls)

                # ---- layers ----
                for l in range(L):
                    with tc.tile_pool(name=f"{pfx}sbe{l}", bufs=3) as sbp, \
                         tc.tile_pool(name=f"{pfx}pse{l}", bufs=2, space="PSUM") as psp, \
                         tc.tile_pool(name=f"{pfx}psg{l}", bufs=2, space="PSUM") as psg:
                        for blk in blocks:
                            b = blk["b"]
                            hr = slice(64, 128) if blk["half"] else slice(0, 64)
                            ba = min(128, ASH - 128 * b)
                            asrc_full = asrc_fulls[l]
                            gt = sbp.tile([128, 1, blk["nblk_e"]], BF16, tag="gt", bufs=2)
                            if blk["n_lo"] and not no_gather:
                                io = sum(bb["n_lo"] for bb in blocks[:b]) // 16
                                nc.gpsimd.dma_gather(
                                    gt[:, :, 0:blk["n_lo"]], asrc_full[0:LOS, :],
                                    idxsb[:, io:io + blk["n_lo"] // 16],
                                    blk["n_lo"], blk["n_lo"], ND, transpose=True,
                                    queue_num=(2 * b) % 4)
                            if blk["n_hi"] and not no_gather:
                                io = sum(bb["n_hi"] for bb in blocks[:b]) // 16
                                nc.gpsimd.dma_gather(
                                    gt[:, :, blk["n_lo"]:], asrc_full[LOS:N, :],
                                    idxsbh[:, io:io + blk["n_hi"] // 16],
                                    blk["n_hi"], blk["n_hi"], ND, transpose=True,
                                    queue_num=(2 * b + 1) % 4)
                            ps_agg = psg.tile([128, 64], F32, tag="agg")
                            nchunk = len(blk["chunks"])
                            e0 = blk["chunks"][0][1]
                            ssb = sbp.tile([128, 2 * blk["nblk_e"]], BF16, tag="ssb", bufs=2)
                            nc.sync.dma_start(ssb[:, 0:2 * blk["nblk_e"]],
                                              ssdev[:, 2 * e0:2 * e0 + 2 * blk["nblk_e"]])
                            for ci, (sco, eco, bco, n) in enumerate(blk["chunks"]):
                                sst = ssb[:, 2 * (eco - e0):2 * (eco - e0) + 2 * n]
                                ps_h = psp.tile([128, 512], F32, tag="ph")
                                nc.tensor.matmul(ps_h[:, 0:n], adst[0:ba, 128 * b:128 * b + 128],
                                                 sst[0:ba, 0:n], start=True, stop=False)  # S^T chunk
                                nc.tensor.matmul(ps_h[:, 0:n], w["we_dup"][hr, l * HID:(l + 1) * HID],
                                                 stateT[hr, sco:sco + n], start=False,
                                                 stop=True)
                                ht = sbp.tile([128, 512], BF16, tag="ht")
                                if not no_gather:
                                    # gathered A_src + eb1 fused on DVE instead of a
                                    # third accumulating matmul on PE
                                    hpre = sbp.tile([128, 512], BF16, tag="hpre")
                                    nc.vector.scalar_tensor_tensor(
                                        hpre[:, 0:n], ps_h[:, 0:n], w["eb1"][:, l:l + 1],
                                        gt[:, 0, bco:bco + n], op0=ALU.add, op1=ALU.add)
                                    nc.scalar.activation(ht[:, 0:n], hpre[:, 0:n], act)
                                else:
                                    nc.scalar.activation(ht[:, 0:n], ps_h[:, 0:n], act,
                                                         bias=w["eb1"][:, l:l + 1])
                                ps_dd = psp.tile([128, 512], F32, tag="pd")
                                nc.tensor.matmul(ps_dd[hr, 0:n], w["ew2_all"][:, l * ED:(l + 1) * ED],
                                                 ht[:, 0:n], start=True, stop=True)
                                # state += mlp_out + eb2 (residual on DVE, not PE)
                                nc.vector.scalar_tensor_tensor(
                                    stateT[hr, sco:sco + n], ps_dd[hr, 0:n],
                                    w["eb2p"][hr, l:l + 1], stateT[hr, sco:sco + n],
                                    op0=ALU.add, op1=ALU.add)
                                ps_t = psp.tile([128, 256], BF16, tag="pt", bufs=1)
                                for j in range(n // 128):
                                    nc.tensor.transpose(
                                        ps_t[:, 64 * j:64 * j + 64],
                                        stateT[hr, sco + 128 * j:sco + 128 * j + 128],
                                        w["i64d"][hr, :])
                                nn = sbp.tile([128, 256], BF16, tag="nn")
                                nc.vector.tensor_copy(nn[:, 0:64 * (n // 128)], ps_t[:, 0:64 * (n // 128)])
                                for j in range(n // 128):
                                    nc.tensor.matmul(
                                        ps_agg[:],
                                        sst[:, n + 128 * j:n + 128 * j + 128],
                                        nn[:, 64 * j:64 * j + 64],
                                        start=(ci == 0 and j == 0),
                                        stop=(ci == nchunk - 1 and j == n // 128 - 1))
                            agnb = sbp.tile([128, 64], BF16, tag="agnb")
                            nc.scalar.activation(agnb[:], ps_agg[:], AFT.Identity,
                                                 scale=invcnt_sb[:, b:b + 1])
                            ps_at = psp.tile([128, 128], BF16, tag="pat", bufs=1)
                            hr2 = slice(64, 128) if b % 2 else slice(0, 64)
                            nc.tensor.transpose(ps_at[hr2, :], agnb[:], w["i128b"][:])
                            nc.vector.tensor_copy(aggT[hr2, (b // 2) * 128:(b // 2) * 128 + 128],
                                                  ps_at[hr2, :])
                    # node MLP + next-layer tables
                    with tc.tile_pool(name=f"{pfx}sbn{l}", bufs=3) as sbp, \
                         tc.tile_pool(name=f"{pfx}psn{l}", bufs=2, space="PSUM") as psp:
                        for t in range(NBLK):
                            na = min(128, ASH - 128 * t)
                            hr2 = slice(64, 128) if t % 2 else slice(0, 64)
                            ps_hn = psp.tile([128, 128], F32, tag="hn")
                            nc.tensor.matmul(ps_hn[:, 0:na],
                                             w["nw1a_all"][:, l * HID:(l + 1) * HID],
                                             nodeTb[:, 128 * t:128 * t + na],
                                             start=True, stop=False)
                            nc.tensor.matmul(ps_hn[:, 0:na],
                                             w["nw1b_dup"][hr2, l * HID:(l + 1) * HID],
                                             aggT[hr2, (t // 2) * 128:(t // 2) * 128 + na],
                                             start=False, stop=True)
                            hn = sbp.tile([128, 128], BF16, tag="hn_s")
                            nc.scalar.activation(hn[:, 0:na], ps_hn[:, 0:na], act,
                                                 bias=w["nb1"][:, l:l + 1])
                            ps_nd = psp.tile([128, 128], F32, tag="ndl")
                            nc.tensor.matmul(ps_nd[:, 0:na],
                                             w["nw2_all"][:, l * ND:(l + 1) * ND],
                                             hn[:, 0:na], start=True, stop=True)
                            nc.vector.scalar_tensor_tensor(
                                nodeT[:, 128 * t:128 * t + na], ps_nd[:, 0:na],
                                w["nb2"][:, l:l + 1], nodeT[:, 128 * t:128 * t + na],
                                op0=ALU.add, op1=ALU.add)
                            nc.vector.tensor_copy(nodeTb[:, 128 * t:128 * t + na],
                                                  nodeT[:, 128 * t:128 * t + na])
                        if l < L - 1:
                            node_tables(l + 1, sbp, psp, asrc_in, asrc_fulls)

                # ---- pooling ----
                with tc.tile_pool(name=pfx + "sbt", bufs=3) as sbt, \
                     tc.tile_pool(name=pfx + "pst", bufs=2, space="PSUM") as pst:
                    for t in range(NBLK):
                        na = min(128, ASH - 128 * t)
                        ps_tr = pst.tile([128, 128], F32, tag="ptr")
                        nc.tensor.transpose(ps_tr[0:na, :], nodeT[:, 128 * t:128 * t + na],
                                            w["i128f"][:])
                        nc.vector.tensor_copy(nnat_all[0:na, 128 * t:128 * t + 128][:, 0:128],
                                              ps_tr[0:na, :])
                with tc.tile_pool(name=pfx + "sbp", bufs=3) as sbp, \
                     tc.tile_pool(name=pfx + "psp", bufs=1, space="PSUM") as psp:
                    cidw_sb = w["cidw"]
                    iota_c = sbp.tile([128, NG * GC], F32, tag="iotac", bufs=1)
                    for q in range(_cdiv(NG * GC, 128)):
                        qn = min(128, NG * GC - 128 * q)
                        nc.vector.tensor_scalar(iota_c[:, 128 * q:128 * q + qn],
                                                w["iotaF"][:, 0:qn], float(128 * q), None,
                                                op0=ALU.add)
                    pools = [psp.tile([128, 128], F32, tag=f"pool{g}", name=f"pool{g}") for g in range(NG)]
                    for t in range(NBLK):
                        na = min(128, ASH - 128 * t)
                        pmt = sbp.tile([128, NG * GC], BF16, tag="pmt")
                        nc.vector.tensor_scalar(pmt[:], iota_c[:], cidw_sb[:, t:t + 1], None,
                                                op0=ALU.is_equal)
                        for g in range(NG):
                            gc = min(GC, NCRYS - g * GC)
                            nc.tensor.matmul(pools[g][0:gc, :], pmt[0:na, g * GC:g * GC + gc],
                                             nnat_all[0:na, 128 * t:128 * t + 128][:, 0:128],
                                             start=(t == 0), stop=(t == NBLK - 1))
                    for g in range(NG):
                        gc = min(GC, NCRYS - g * GC)
                        pev = sbp.tile([128, 128], F32, tag="pev")
                        nc.vector.tensor_copy(pev[0:gc, :], pools[g][0:gc, :])
                        nc.sync.dma_start(pool_in[g * GC:g * GC + gc, :], pev[0:gc, :])
                    if not no_coll:
                        nc.gpsimd.collective_compute(
                            "AllReduce", mybir.AluOpType.add,
                            replica_groups=[list(range(NCORES))],
                            ins=[pool_in[:].opt()], outs=[pool_out[:].opt()],
                        )
                    else:
                        nc.sync.dma_start(pool_out[:], pool_in[:])

                # ---- readout (replicated) ----
                with tc.tile_pool(name=pfx + "sbr", bufs=2) as sbp, \
                     tc.tile_pool(name=pfx + "psr", bufs=2, space="PSUM") as psp:
                    for g in range(NG):
                        gc = min(GC, NCRYS - g * GC)
                        pg = sbp.tile([128, 128], F32, tag="pg")
                        nc.sync.dma_start(pg[0:gc, :], pool_out[g * GC:g * GC + gc, :])
                        mean = sbp.tile([128, 128], BF16, tag="mean")
                        nc.scalar.activation(mean[0:gc, :], pg[0:gc, :], AFT.Identity,
                                             scale=w["invccnt"][0:gc, g:g + 1])
                        ps_mt = psp.tile([128, 128], BF16, tag="pmt2")
                        nc.tensor.transpose(ps_mt[:, 0:gc], mean[0:gc, :], w["i128b"][0:gc, 0:gc])
                        meanT = sbp.tile([128, 128], BF16, tag="meanT")
                        nc.vector.tensor_copy(meanT[:, 0:gc], ps_mt[:, 0:gc])
                        ps_hr = psp.tile([128, 128], F32, tag="phr")
                        nc.tensor.matmul(ps_hr[:, 0:gc], w["readW"][:], meanT[:, 0:gc],
                                         start=True, stop=True)
                        hrT = sbp.tile([128, 128], BF16, tag="hrT")
                        nc.scalar.activation(hrT[:, 0:gc], ps_hr[:, 0:gc], act,
                                             bias=w["readb"][:])
                        ps_y = psp.tile([128, 128], F32, tag="py")
                        nc.tensor.matmul(ps_y[0:1, 0:gc], w["outW"][:], hrT[:, 0:gc],
                                         start=True, stop=True)
                        ysb = sbp.tile([1, 128], F32, tag="ysb")
                        nc.scalar.activation(ysb[0:1, 0:gc], ps_y[0:1, 0:gc], AFT.Copy,
                                             bias=meta["out_b"])
                        nc.sync.dma_start(y[0:1, g * GC:g * GC + gc], ysb[0:1, 0:gc])

            for rep in range(repeat):
                emit_once(f"r{rep}_" if repeat > 1 else "")

    nc.compile()
    return nc


def run_cores(meta, in_maps, act=AFT.Silu, sim=False):
    nc = _build(meta, act=act)
    if sim:
        from concourse.bass_interp import MultiCoreSim
        s = MultiCoreSim(nc, NCORES, trace=False)
        for k in range(NCORES):
            for nm, arr in in_maps[k].items():
                s.cores[k].tensor(nm)[:] = arr
        s.simulate(check_with_hw=False)
        return [{"y": np.array(s.cores[k].tensor("y"))} for k in range(NCORES)], None
    from concourse import bass_utils
    res = bass_utils.run_bass_kernel_spmd(nc, in_maps, core_ids=list(range(NCORES)))
    return res.results, res


def kernel(**inputs):
    cfg = dict(FULL_CFG)
    n, m = np.asarray(inputs["nbr_fea_idx"]).shape
    cfg["N"], cfg["M"] = int(n), int(m)
    cfg["AFD"] = int(np.asarray(inputs["atom_fea"]).shape[1])
    cfg["EFD"] = int(np.asarray(inputs["nbr_fea"]).shape[2])
    cfg["NCRYS"] = int(inputs["num_crystals"])
    cfg["L"] = int(np.asarray(inputs["eW1"]).shape[0])
    meta, in_maps = _prep(inputs, cfg)
    results, _ = run_cores(meta, in_maps)
    return np.asarray(results[0]["y"], np.float32).reshape(cfg["NCRYS"], 1)


# revision 14
# speedup vs baseline: 25.1058x; 1.0596x over previous
"""CrystalGraphALIGNN Trainium2 kernel (8 NeuronCores, SPMD).

Strategy: dst-shard edges across cores (atom v owned by core v // (N/8); edge
(i,j) owned by the core of its dst). Per core, edges are sorted by dst and
grouped into 128-atom blocks so that:
  - the dst-side expansion A_dst[dst(e)] is a block-local one-hot matmul,
  - the scatter-mean aggregation is a one-hot matmul into PSUM,
  - only the src side needs a true random gather: per-edge rows of
    A_src = node @ W_src, fetched with dma_gather (transposed, bf16) from a
    DRAM table that is refreshed once per layer via AllGather.
The block-local one-hot matrices are never shipped from the host: the host
sends one bf16/f32 row index per edge slot and the kernel expands them into
a DRAM scratch at init with a rank-1 broadcast matmul + is_equal compare.
Node states and the node MLP stay fully shard-local; crystal pooling is a
one-hot matmul (also built on device from per-atom crystal ids) + a single
AllReduce, readout replicated on every core.

_build(repeat=R) emits the whole computation R times in one program (weights
loaded once); timing builds with different R and taking the delta cancels
the per-call host/tunnel dispatch overhead, which is how test.py measures
the hardware execution time without NTFF profiling.
"""

import numpy as np
import ml_dtypes

import concourse.bass as bass
import concourse.bacc as bacc
import concourse.mybir as mybir
import concourse.tile as tile
from concourse import library_config

F32 = mybir.dt.float32
BF16 = mybir.dt.bfloat16
F8 = mybir.dt.float8e4
I16 = mybir.dt.int16
AFT = mybir.ActivationFunctionType
ALU = mybir.AluOpType
BF = ml_dtypes.bfloat16
F8E4 = ml_dtypes.float8_e4m3

NCORES = 8
ED, ND, HID, RD = 64, 128, 128, 128
EDGE_THRESH = 1e-6
GC = 125  # crystals per pooling group

FULL_CFG = dict(N=50000, M=12, AFD=92, EFD=41, NCRYS=1000, L=4)


def _cdiv(a, b):
    return (a + b - 1) // b


def _wrap_idx(flat):
    """int16 flat idx [n] -> [16, n/16] wrapped layout (replicated on device)."""
    n = flat.shape[-1]
    assert n % 16 == 0
    return np.ascontiguousarray(flat.reshape(n // 16, 16).T.astype(np.int16))


def _prep(inputs, cfg):
    N, M, AFD, EFD, NCRYS, L = (cfg[k] for k in ("N", "M", "AFD", "EFD", "NCRYS", "L"))
    ASH = N // NCORES
    NBLK = _cdiv(ASH, 128)
    LOS = min(25000, N)  # src index split for int16 gather indices
    NG = _cdiv(NCRYS, GC)

    af = np.asarray(inputs["atom_fea"], np.float32)
    nf = np.asarray(inputs["nbr_fea"], np.float32)
    nidx = np.asarray(inputs["nbr_fea_idx"]).astype(np.int64)
    cb = np.asarray(inputs["crystal_batch"]).astype(np.int64)

    E = N * M
    dst = np.clip(nidx.reshape(-1), 0, N - 1)
    src = np.repeat(np.arange(N, dtype=np.int64), M)
    ea = nf.reshape(E, EFD)
    mask = (np.abs(ea).sum(1) > EDGE_THRESH).astype(np.float32)

    cnt = np.bincount(dst, weights=mask, minlength=N)
    invcnt = (1.0 / np.maximum(cnt, 1.0)).astype(np.float32)
    ccnt = np.bincount(cb, minlength=NCRYS).astype(np.float32)
    invccnt = (1.0 / np.maximum(ccnt, 1.0)).astype(np.float32)

    core_of = dst // ASH
    dloc = dst - core_of * ASH
    blk_of = dloc // 128
    arow = dloc - blk_of * 128
    half = (src >= LOS).astype(np.int64)

    # per-(core, block, half) edge counts -> shared tile geometry
    key = (core_of * NBLK + blk_of) * 2 + half
    cnts = np.bincount(key, minlength=NCORES * NBLK * 2).reshape(NCORES, NBLK, 2)
    T_lo = np.maximum((cnts[:, :, 0].max(0) + 127) // 128, 1)
    T_hi = (cnts[:, :, 1].max(0) + 127) // 128
    odd = ((T_lo + T_hi) % 2).astype(np.int64)
    if N > LOS:
        T_hi = T_hi + odd
    else:
        T_lo = T_lo + odd
    n_lo = T_lo * 128
    n_hi = T_hi * 128
    nblk_e = n_lo + n_hi
    ecol = np.zeros(NBLK + 1, np.int64)
    ecol[1:] = np.cumsum(nblk_e)
    EP = int(ecol[NBLK])

    # geometry: edge-col space (block-major), state-col space (per half),
    # chunk list entries: (state_col, edge_col, blk_edge_col, n)
    BHALF = NBLK // 2
    scol = np.zeros(NBLK, np.int64)
    acc = [0, 0]
    blocks = []
    for b in range(NBLK):
        hb = 0 if b < BHALF else 1
        scol[b] = acc[hb]
        acc[hb] += int(nblk_e[b])
        tiles = int(T_lo[b] + T_hi[b])
        chunks = []
        off = 0
        while tiles > 0:
            t = 4 if tiles >= 4 else tiles
            chunks.append((int(scol[b] + off), int(ecol[b] + off), off, t * 128))
            off += t * 128
            tiles -= t
        blocks.append(dict(b=b, half=hb, nblk_e=int(nblk_e[b]), chunks=chunks,
                           n_lo=int(n_lo[b]), n_hi=int(n_hi[b])))
    EPC = max(acc)
    IWL = int(T_lo.sum()) * 8
    IWH = int(T_hi.sum()) * 8

    meta = dict(cfg=cfg, ASH=ASH, NBLK=NBLK, LOS=LOS, NG=NG, EP=EP, EPC=EPC,
                BHALF=BHALF, blocks=blocks, IWL=IWL, IWH=IWH,
                out_b=float(np.asarray(inputs["out_b"]).reshape(-1)[0]))

    # ---- vectorized edge -> slot assignment ----
    order = np.argsort(key, kind="stable")  # edges grouped by (core, blk, half)
    starts = np.zeros(NCORES * NBLK * 2 + 1, np.int64)
    starts[1:] = np.cumsum(cnts.reshape(-1))
    ks = key[order]
    rank = np.arange(E, dtype=np.int64) - starts[ks]
    blk_s = blk_of[order]
    slot = ecol[blk_s] + half[order] * n_lo[blk_s] + rank
    gslot = core_of[order] * EP + slot
    perm = np.full(NCORES * EP, -1, np.int64)
    perm[gslot] = order
    valid = perm >= 0
    pc = np.where(valid, perm, 0)

    # permuted edge features (bias folded in on device via per-partition add)
    eat = ea[pc]
    eat[~valid] = 0.0
    eat = eat.astype(BF).reshape(NCORES, EP, EFD)

    # per-slot one-hot row indices: 255 -> no match -> zero column/row
    arow_s = arow[pc]
    rexp_all = np.where(valid, arow_s, 255).astype(BF).reshape(NCORES, 1, EP)
    ragg_all = np.where(valid & (mask[pc] > 0), arow_s, 255).astype(np.float32)
    raggw_all = np.ascontiguousarray(
        ragg_all.reshape(NCORES, EP // 128, 128).transpose(0, 2, 1))

    # gather indices (src node ids) for lo/hi slot regions
    blkslot = np.repeat(np.arange(NBLK), nblk_e)
    islo = (np.arange(EP) - ecol[blkslot]) < n_lo[blkslot]
    gsrc = np.where(valid, src[pc], 0).reshape(NCORES, EP)
    ghi = np.where(valid, src[pc] - LOS, 0).reshape(NCORES, EP)
    lo_idx = gsrc[:, islo]
    hi_idx = ghi[:, ~islo]

    # shared weights
    eW1 = np.asarray(inputs["eW1"], np.float32)
    eW2 = np.asarray(inputs["eW2"], np.float32)
    nW1 = np.asarray(inputs["nW1"], np.float32)
    nW2 = np.asarray(inputs["nW2"], np.float32)

    def bfc(x):
        return np.ascontiguousarray(x, np.float32).astype(BF)

    atomW93 = np.zeros((AFD + 1, ND), np.float32)
    atomW93[:AFD] = np.asarray(inputs["atom_W"], np.float32)
    atomW93[AFD] = np.asarray(inputs["atom_b"], np.float32)

    we_dup = np.zeros((128, L * HID), np.float32)
    nw1b_dup = np.zeros((128, L * ND), np.float32)
    for l in range(L):
        we_dup[0:64, l * HID:(l + 1) * HID] = eW1[l, 0:ED]
        we_dup[64:128, l * HID:(l + 1) * HID] = eW1[l, 0:ED]
        nw1b_dup[0:64, l * ND:(l + 1) * ND] = nW1[l, ND:ND + ED]
        nw1b_dup[64:128, l * ND:(l + 1) * ND] = nW1[l, ND:ND + ED]
    # ws|wd packed per layer so node_tables does one matmul per block
    wswd = np.concatenate(
        [np.concatenate([eW1[l, ED:ED + ND], eW1[l, ED + ND:]], 1) for l in range(L)], 1)
    ew2_all = np.concatenate([eW2[l] for l in range(L)], 1)                 # [128, L*64]
    nw1a_all = np.concatenate([nW1[l, 0:ND] for l in range(L)], 1)          # [128, L*128]
    nw2_all = np.concatenate([nW2[l] for l in range(L)], 1)                 # [128, L*128]

    eb1 = np.asarray(inputs["eb1"], np.float32).T.copy()                    # [128, L]
    eb2p = np.zeros((128, L), np.float32)
    eb2p[0:64] = np.asarray(inputs["eb2"], np.float32).T
    eb2p[64:128] = eb2p[0:64]
    nb1 = np.asarray(inputs["nb1"], np.float32).T.copy()
    nb2 = np.asarray(inputs["nb2"], np.float32).T.copy()
    edgeb2 = np.zeros((128, 1), np.float32)
    edgeb2[0:64, 0] = np.asarray(inputs["edge_b"], np.float32)
    edgeb2[64:128, 0] = edgeb2[0:64, 0]

    i64d = np.zeros((128, 64), np.float32)
    i64d[0:64] = np.eye(64)
    i64d[64:128] = np.eye(64)

    # ---- pack all weights/constants into two buffers (per-tensor transfer
    # cost through the tunnel is ~tens of ms, so fewer tensors = faster) ----
    EPT = EP // 128
    WB = [("atomW", AFD + 1, ND), ("we_dup", 128, L * HID),
          ("nw1b_dup", 128, L * ND), ("wswd", 128, L * 256), ("ew2_all", HID, L * ED),
          ("nw1a_all", ND, L * HID), ("nw2_all", HID, L * ND), ("readW", ND, RD),
          ("outW", RD, 1), ("i64d", 128, 64), ("i128b", 128, 128)]
    WF = [("eb1", 128, L), ("eb2p", 128, L), ("nb1", 128, L), ("nb2", 128, L),
          ("readb", RD, 1), ("edgeb2", 128, 1), ("i128f", 128, 128),
          ("iotaF", 128, 128), ("iotap", 128, 1), ("invccnt", GC, NG),
          ("invcnt", 128, NBLK), ("cidw", 128, NBLK), ("raggw", 128, EPT)]

    def offsets(entries):
        offs, off = {}, 0
        for nm, r, c in entries:
            offs[nm] = (r, c, off)
            off += c
        return offs, off

    WBO, XB = offsets(WB)
    WFO, XF = offsets(WF)
    meta.update(WBO=WBO, WFO=WFO, XB=XB, XF=XF)

    vals_b = {
        "atomW": bfc(atomW93),
        "we_dup": bfc(we_dup), "nw1b_dup": bfc(nw1b_dup), "wswd": bfc(wswd),
        "ew2_all": bfc(ew2_all), "nw1a_all": bfc(nw1a_all), "nw2_all": bfc(nw2_all),
        "readW": bfc(np.asarray(inputs["read_W"])), "outW": bfc(np.asarray(inputs["out_W"])),
        "i64d": bfc(i64d), "i128b": bfc(np.eye(128)),
    }
    vals_f = {
        "eb1": eb1, "eb2p": eb2p, "nb1": nb1, "nb2": nb2, "edgeb2": edgeb2,
        "readb": np.asarray(inputs["read_b"], np.float32).reshape(RD, 1),
        "i128f": np.eye(128, dtype=np.float32),
        "iotaF": np.tile(np.arange(128, dtype=np.float32), (128, 1)),
        "iotap": np.arange(128, dtype=np.float32).reshape(128, 1),
        "invccnt": np.pad(invccnt, (0, NG * GC - NCRYS)).reshape(NG, GC).T.copy(),
    }
    wb = np.zeros((128, XB), BF)
    for nm, (r, c, off) in WBO.items():
        wb[0:r, off:off + c] = vals_b[nm]

    in_maps = []
    for k in range(NCORES):
        a0 = k * ASH
        inv_sb = np.ones((128, NBLK), np.float32)
        cidw = np.full((128, NBLK), -1.0, np.float32)
        for b in range(NBLK):
            na = min(128, ASH - 128 * b)
            inv_sb[0:na, b] = invcnt[a0 + 128 * b: a0 + 128 * b + na]
            cidw[0:na, b] = cb[a0 + 128 * b: a0 + 128 * b + na]
        afT = np.zeros((AFD + 1, ASH), np.float32)
        afT[:AFD] = af[a0:a0 + ASH].T
        afT[AFD] = 1.0
        wf = np.zeros((128, XF), np.float32)
        for nm, (r, c, off) in WFO.items():
            v = vals_f.get(nm)
            if nm == "invcnt":
                v = inv_sb
            elif nm == "cidw":
                v = cidw
            elif nm == "raggw":
                v = raggw_all[k]
            wf[0:r, off:off + c] = v

        m = {
            # edge features + edge_W quantized to fp8e4m3 (features are ~N(0,1);
            # ~3% element error stays far inside the accuracy budget)
            "eat": np.concatenate(
                [np.ascontiguousarray(eat[k].T),
                 np.asarray(inputs["edge_W"], np.float32)], 1).astype(F8E4),
            "rexp": rexp_all[k],
            "idxs": np.concatenate(
                [_wrap_idx(lo_idx[k]), _wrap_idx(hi_idx[k])], 1) if IWH
                else _wrap_idx(lo_idx[k]),
            "afT": afT.astype(BF), "wb": wb, "wf": wf,
        }
        in_maps.append(m)
    return meta, in_maps


def _build(meta, act=AFT.Silu, noop=False, no_gather=False, no_coll=False,
           repeat=1, skip=()):
    skip = set(skip)
    if no_gather:
        skip.add("gather")
    if no_coll:
        skip.add("coll")
    cfg = meta["cfg"]
    N, M, AFD, EFD, NCRYS, L = (cfg[k] for k in ("N", "M", "AFD", "EFD", "NCRYS", "L"))
    ASH, NBLK, LOS, NG = meta["ASH"], meta["NBLK"], meta["LOS"], meta["NG"]
    EP, EPC, blocks = meta["EP"], meta["EPC"], meta["blocks"]
    IWL, IWH = meta["IWL"], meta["IWH"]
    EPT = EP // 128

    WBO, WFO, XB, XF = meta["WBO"], meta["WFO"], meta["XB"], meta["XF"]

    nc = bacc.Bacc("TRN2", target_bir_lowering=False, debug=False, num_devices=NCORES,
                   num_swdge_queues=4)

    def din(name, shape, dt):
        return nc.dram_tensor(name, shape, dt, kind="ExternalInput")

    eat_d = din("eat", [EFD, EP + ED], F8)       # fp8 edge features | edge_W
    rexp_d = din("rexp", [1, EP], BF16)
    idxs_d = din("idxs", [16, IWL + IWH], I16)
    afT = din("afT", [AFD + 1, ASH], BF16)
    wb_d = din("wb", [128, XB], BF16)
    wf_d = din("wf", [128, XF], F32)
    y = nc.dram_tensor("y", [1, NCRYS], F32, kind="ExternalOutput")

    if noop:
        with tile.TileContext(nc) as tc:
            with tc.tile_pool(name="sbz", bufs=1) as sbz:
                yz = sbz.tile([1, NCRYS], F32, tag="yz")
                nc.gpsimd.memset(yz[:], 0.0)
                nc.sync.dma_start(y[:], yz[:])
        nc.compile()
        return nc

    with tile.TileContext(nc) as tc:
        with (
            tc.tile_pool(name="persist", bufs=1) as pp,
            tc.tile_pool(name="dram", bufs=1, space="DRAM") as dp,
        ):
            nc.gpsimd.load_library(library_config.mlp)
            w = {}
            for nm, (r, c, off) in WBO.items():
                w[nm] = pp.tile([r, c], BF16, tag=nm, name=f"w_{nm}")
                nc.sync.dma_start(w[nm][:], wb_d[0:r, off:off + c])
            for nm, (r, c, off) in WFO.items():
                if nm == "raggw":
                    continue  # init-only; loaded into the init pool below
                w[nm] = pp.tile([r, c], F32, tag=nm, name=f"w_{nm}")
                nc.sync.dma_start(w[nm][:], wf_d[0:r, off:off + c])
            w["edgeW"] = pp.tile([EFD, ED], F8, tag="edgeW", name="w_edgeW")
            nc.sync.dma_start(w["edgeW"][:], eat_d[:, EP:EP + ED])
            invcnt_sb = w["invcnt"]
            stateT = pp.tile([128, EPC], BF16, tag="stateT")
            nodeT = pp.tile([128, ASH], F32, tag="nodeT")
            nodeTb = pp.tile([128, ASH], BF16, tag="nodeTb")
            adst = pp.tile([128, NBLK * 128], BF16, tag="adst")
            aggT = pp.tile([128, _cdiv(NBLK, 2) * 128], BF16, tag="aggT")
            nnat_all = pp.tile([128, NBLK * 128], BF16, tag="nnat_all")
            ones1 = pp.tile([1, 128], BF16, tag="ones1")
            nc.vector.memset(ones1[:], 1.0)
            idxsb = pp.tile([128, IWL], I16, tag="idxsb")
            for r in range(8):
                nc.sync.dma_start(idxsb[16 * r:16 * r + 16, :], idxs_d[:, 0:IWL])
            if IWH:
                idxsbh = pp.tile([128, IWH], I16, tag="idxsbh")
                for r in range(8):
                    nc.sync.dma_start(idxsbh[16 * r:16 * r + 16, :], idxs_d[:, IWL:IWL + IWH])
            ssdev = dp.tile([128, 2 * EP], BF16)  # device-built one-hot scatter mats
            if "ssb" in skip:
                ssbd = pp.tile([128, 2 * max(bl["nblk_e"] for bl in blocks)],
                               BF16, tag="ssbd")
                nc.vector.memset(ssbd[:], 0.0)
            if "agg" in skip:
                nc.vector.memset(aggT[:], 0.0)

            def node_tables(lw, sbp, psp, asrc_in, asrc_fulls):
                """A_src shard -> bounce -> AllGather; A_dst blocks (layer lw)."""
                for t in range(NBLK):
                    na = min(128, ASH - 128 * t)
                    lhs = nodeTb[:, 128 * t:128 * t + na]
                    ps_s = psp.tile([128, 256], F32, tag="ps_s")
                    nc.tensor.matmul(ps_s[0:na, :], lhs, w["wswd"][:, lw * 256:(lw + 1) * 256],
                                     start=True, stop=True)
                    asb = sbp.tile([128, 128], BF16, tag="asb")
                    nc.vector.tensor_copy(asb[0:na, :], ps_s[0:na, 0:128])
                    nc.sync.dma_start(asrc_in[128 * t:128 * t + na, :], asb[0:na, :])
                    nc.vector.tensor_copy(adst[0:na, 128 * t:128 * t + 128][:, 0:128],
                                          ps_s[0:na, 128:256])
                if "coll" not in skip:
                    nc.gpsimd.collective_compute(
                        "AllGather", mybir.AluOpType.bypass,
                        replica_groups=[list(range(NCORES))],
                        ins=[asrc_in[:].opt()], outs=[asrc_fulls[lw][:].opt()],
                    )
                else:
                    nc.sync.dma_start(asrc_fulls[lw][0:ASH, :], asrc_in[:])

            def emit_once(pfx):
                # collective buffers are per-rep: Shared DRAM outputs may only
                # have a single writer instruction
                asrc_in = dp.tile([ASH, ND], BF16, tag=f"{pfx}asrc_in")
                asrc_fulls = [dp.tile([N, ND], BF16, addr_space="Shared",
                                      name=f"{pfx}asrc_full{i}", tag=f"{pfx}asrc_full{i}")
                              for i in range(L)]
                pool_in = dp.tile([NCRYS, ND], F32, tag=f"{pfx}pool_in")
                pool_out = dp.tile([NCRYS, ND], F32, addr_space="Shared",
                                   tag=f"{pfx}pool_out")
                # ---- init: projections + device-side one-hot build + layer-0 tables ----
                with tc.tile_pool(name=pfx + "sbi", bufs=3) as sbp, \
                     tc.tile_pool(name=pfx + "psi", bufs=2, space="PSUM") as psp:
                    rr, rc, roff = WFO["raggw"]
                    raggw_sb = sbp.tile([128, EPT], F32, tag="raggw_sb", bufs=1)
                    nc.sync.dma_start(raggw_sb[:], wf_d[0:rr, roff:roff + rc])
                    for t in range(NBLK):
                        na = min(128, ASH - 128 * t)
                        aft = sbp.tile([AFD + 1, 128], BF16, tag="aft")
                        nc.sync.dma_start(aft[:, 0:na], afT[:, 128 * t:128 * t + na])
                        ps_n = psp.tile([128, 128], F32, tag="ps_n")
                        nc.tensor.matmul(ps_n[:, 0:na], w["atomW"][:], aft[:, 0:na],
                                         start=True, stop=True)
                        nc.vector.tensor_copy(nodeT[:, 128 * t:128 * t + na], ps_n[:, 0:na])
                        nc.vector.tensor_copy(nodeTb[:, 128 * t:128 * t + na], ps_n[:, 0:na])
                    for blk in blocks:
                        hr = slice(64, 128) if blk["half"] else slice(0, 64)
                        for (sco, eco, bco, n) in blk["chunks"]:
                            eat = sbp.tile([EFD, 512], F8, tag="eat")
                            nc.sync.dma_start(eat[:, 0:n], eat_d[0:EFD, eco:eco + n])
                            ps_e = psp.tile([128, 512], F32, tag="ps_e")
                            nc.tensor.matmul(ps_e[hr, 0:n], w["edgeW"][:], eat[:, 0:n],
                                             start=True, stop=True)
                            nc.vector.tensor_scalar(stateT[hr, sco:sco + n], ps_e[hr, 0:n],
                                                    w["edgeb2"][hr, 0:1], None, op0=ALU.add)
                            if "init_onehot" in skip:
                                continue
                            # expand per-slot row ids into one-hot scatter mats -> DRAM
                            rx = sbp.tile([1, 512], BF16, tag="rx")
                            nc.sync.dma_start(rx[0:1, 0:n], rexp_d[0:1, eco:eco + n])
                            ps_b = psp.tile([128, 512], F32, tag="ps_e")
                            nc.tensor.matmul(ps_b[:, 0:n], ones1[0:1, :], rx[0:1, 0:n],
                                             start=True, stop=True)
                            sst = sbp.tile([128, 1024], BF16, tag="sst")
                            nc.vector.tensor_scalar(sst[:, 0:n], ps_b[:, 0:n],
                                                    w["iotap"][:, 0:1], None, op0=ALU.is_equal)
                            g0 = eco // 128
                            for j in range(n // 128):
                                nc.vector.tensor_scalar(sst[:, n + 128 * j:n + 128 * j + 128],
                                                        w["iotaF"][:],
                                                        raggw_sb[:, g0 + j:g0 + j + 1], None,
                                                        op0=ALU.is_equal)
                            nc.sync.dma_start(ssdev[:, 2 * eco:2 * eco + 2 * n], sst[:, 0:2 * n])
                    node_tables(0, sbp, psp, asrc_in, asrc_fulls)

                # ---- layers ----
                for l in range(L):
                    with tc.tile_pool(name=f"{pfx}sbe{l}", bufs=3) as sbp, \
                         tc.tile_pool(name=f"{pfx}pse{l}", bufs=2, space="PSUM") as psp, \
                         tc.tile_pool(name=f"{pfx}psg{l}", bufs=2, space="PSUM") as psg:
                        for blk in blocks:
                            b = blk["b"]
                            hr = slice(64, 128) if blk["half"] else slice(0, 64)
                            ba = min(128, ASH - 128 * b)
                            asrc_full = asrc_fulls[l]
                            gt = sbp.tile([128, 1, blk["nblk_e"]], BF16, tag="gt", bufs=2)
                            if blk["n_lo"] and "gather" not in skip:
                                io = sum(bb["n_lo"] for bb in blocks[:b]) // 16
                                nc.gpsimd.dma_gather(
                                    gt[:, :, 0:blk["n_lo"]], asrc_full[0:LOS, :],
                                    idxsb[:, io:io + blk["n_lo"] // 16],
                                    blk["n_lo"], blk["n_lo"], ND, transpose=True,
                                    queue_num=(2 * b) % 4)
                            if blk["n_hi"] and "gather" not in skip:
                                io = sum(bb["n_hi"] for bb in blocks[:b]) // 16
                                nc.gpsimd.dma_gather(
                                    gt[:, :, blk["n_lo"]:], asrc_full[LOS:N, :],
                                    idxsbh[:, io:io + blk["n_hi"] // 16],
                                    blk["n_hi"], blk["n_hi"], ND, transpose=True,
                                    queue_num=(2 * b + 1) % 4)
                            do_agg = "agg" not in skip
                            if do_agg:
                                ps_agg = psg.tile([128, 64], F32, tag="agg")
                            nchunk = len(blk["chunks"])
                            e0 = blk["chunks"][0][1]
                            if "ssb" not in skip:
                                ssb = sbp.tile([128, 2 * blk["nblk_e"]], BF16, tag="ssb", bufs=2)
                                nc.sync.dma_start(ssb[:, 0:2 * blk["nblk_e"]],
                                                  ssdev[:, 2 * e0:2 * e0 + 2 * blk["nblk_e"]])
                            else:
                                ssb = ssbd
                            for ci, (sco, eco, bco, n) in enumerate(blk["chunks"]):
                                sst = ssb[:, 2 * (eco - e0):2 * (eco - e0) + 2 * n]
                                if "edge_mm" not in skip:
                                    ps_h = psp.tile([128, 512], F32, tag="ph")
                                    nc.tensor.matmul(ps_h[:, 0:n], adst[0:ba, 128 * b:128 * b + 128],
                                                     sst[0:ba, 0:n], start=True, stop=False)  # S^T chunk
                                    nc.tensor.matmul(ps_h[:, 0:n], w["we_dup"][hr, l * HID:(l + 1) * HID],
                                                     stateT[hr, sco:sco + n], start=False,
                                                     stop=True)
                                    ht = sbp.tile([128, 512], BF16, tag="ht")
                                    if "gather" not in skip:
                                        # gathered A_src + eb1 fused on DVE instead of a
                                        # third accumulating matmul on PE
                                        hpre = sbp.tile([128, 512], BF16, tag="hpre")
                                        nc.vector.scalar_tensor_tensor(
                                            hpre[:, 0:n], ps_h[:, 0:n], w["eb1"][:, l:l + 1],
                                            gt[:, 0, bco:bco + n], op0=ALU.add, op1=ALU.add)
                                        nc.scalar.activation(ht[:, 0:n], hpre[:, 0:n], act)
                                    else:
                                        nc.scalar.activation(ht[:, 0:n], ps_h[:, 0:n], act,
                                                             bias=w["eb1"][:, l:l + 1])
                                    ps_dd = psp.tile([128, 512], F32, tag="pd")
                                    nc.tensor.matmul(ps_dd[hr, 0:n], w["ew2_all"][:, l * ED:(l + 1) * ED],
                                                     ht[:, 0:n], start=True, stop=True)
                                    # state += mlp_out + eb2 (residual on DVE, not PE)
                                    nc.vector.scalar_tensor_tensor(
                                        stateT[hr, sco:sco + n], ps_dd[hr, 0:n],
                                        w["eb2p"][hr, l:l + 1], stateT[hr, sco:sco + n],
                                        op0=ALU.add, op1=ALU.add)
                                if not do_agg:
                                    continue
                                ps_t = psp.tile([128, 256], BF16, tag="pt", bufs=1)
                                for j in range(n // 128):
                                    nc.tensor.transpose(
                                        ps_t[:, 64 * j:64 * j + 64],
                                        stateT[hr, sco + 128 * j:sco + 128 * j + 128],
                                        w["i64d"][hr, :])
                                nn = sbp.tile([128, 256], BF16, tag="nn")
                                nc.vector.tensor_copy(nn[:, 0:64 * (n // 128)], ps_t[:, 0:64 * (n // 128)])
                                for j in range(n // 128):
                                    nc.tensor.matmul(
                                        ps_agg[:],
                                        sst[:, n + 128 * j:n + 128 * j + 128],
                                        nn[:, 64 * j:64 * j + 64],
                                        start=(ci == 0 and j == 0),
                                        stop=(ci == nchunk - 1 and j == n // 128 - 1))
                            if not do_agg:
                                continue
                            agnb = sbp.tile([128, 64], BF16, tag="agnb")
                            nc.scalar.activation(agnb[:], ps_agg[:], AFT.Identity,
                                                 scale=invcnt_sb[:, b:b + 1])
                            ps_at = psp.tile([128, 128], BF16, tag="pat", bufs=1)
                            hr2 = slice(64, 128) if b % 2 else slice(0, 64)
                            nc.tensor.transpose(ps_at[hr2, :], agnb[:], w["i128b"][:])
                            nc.vector.tensor_copy(aggT[hr2, (b // 2) * 128:(b // 2) * 128 + 128],
                                                  ps_at[hr2, :])
                    # node MLP + next-layer tables
                    with tc.tile_pool(name=f"{pfx}sbn{l}", bufs=3) as sbp, \
                         tc.tile_pool(name=f"{pfx}psn{l}", bufs=2, space="PSUM") as psp:
                        for t in range(NBLK if "node" not in skip else 0):
                            na = min(128, ASH - 128 * t)
                            hr2 = slice(64, 128) if t % 2 else slice(0, 64)
                            ps_hn = psp.tile([128, 128], F32, tag="hn")
                            nc.tensor.matmul(ps_hn[:, 0:na],
                                             w["nw1a_all"][:, l * HID:(l + 1) * HID],
                                             nodeTb[:, 128 * t:128 * t + na],
                                             start=True, stop=False)
                            nc.tensor.matmul(ps_hn[:, 0:na],
                                             w["nw1b_dup"][hr2, l * HID:(l + 1) * HID],
                                             aggT[hr2, (t // 2) * 128:(t // 2) * 128 + na],
                                             start=False, stop=True)
                            hn = sbp.tile([128, 128], BF16, tag="hn_s")
                            nc.scalar.activation(hn[:, 0:na], ps_hn[:, 0:na], act,
                                                 bias=w["nb1"][:, l:l + 1])
                            ps_nd = psp.tile([128, 128], F32, tag="ndl")
                            nc.tensor.matmul(ps_nd[:, 0:na],
                                             w["nw2_all"][:, l * ND:(l + 1) * ND],
                                             hn[:, 0:na], start=True, stop=True)
                            nc.vector.scalar_tensor_tensor(
                                nodeT[:, 128 * t:128 * t + na], ps_nd[:, 0:na],
                                w["nb2"][:, l:l + 1], nodeT[:, 128 * t:128 * t + na],
                                op0=ALU.add, op1=ALU.add)
                            nc.vector.tensor_copy(nodeTb[:, 128 * t:128 * t + na],
                                                  nodeT[:, 128 * t:128 * t + na])
                        if l < L - 1:
                            node_tables(l + 1, sbp, psp, asrc_in, asrc_fulls)

                # ---- pooling ----
                if "pool" in skip:
                    with tc.tile_pool(name=pfx + "sbz", bufs=1) as sbz:
                        yz = sbz.tile([1, NCRYS], F32, tag="yz")
                        nc.gpsimd.memset(yz[:], 0.0)
                        nc.sync.dma_start(y[:], yz[:])
                    return
                with tc.tile_pool(name=pfx + "sbt", bufs=3) as sbt, \
                     tc.tile_pool(name=pfx + "pst", bufs=2, space="PSUM") as pst:
                    for t in range(NBLK):
                        na = min(128, ASH - 128 * t)
                        ps_tr = pst.tile([128, 128], F32, tag="ptr")
                        nc.tensor.transpose(ps_tr[0:na, :], nodeT[:, 128 * t:128 * t + na],
                                            w["i128f"][:])
                        nc.vector.tensor_copy(nnat_all[0:na, 128 * t:128 * t + 128][:, 0:128],
                                              ps_tr[0:na, :])
                with tc.tile_pool(name=pfx + "sbp", bufs=3) as sbp, \
                     tc.tile_pool(name=pfx + "psp", bufs=1, space="PSUM") as psp:
                    cidw_sb = w["cidw"]
                    iota_c = sbp.tile([128, NG * GC], F32, tag="iotac", bufs=1)
                    for q in range(_cdiv(NG * GC, 128)):
                        qn = min(128, NG * GC - 128 * q)
                        nc.vector.tensor_scalar(iota_c[:, 128 * q:128 * q + qn],
                                                w["iotaF"][:, 0:qn], float(128 * q), None,
                                                op0=ALU.add)
                    pools = [psp.tile([128, 128], F32, tag=f"pool{g}", name=f"pool{g}") for g in range(NG)]
                    for t in range(NBLK):
                        na = min(128, ASH - 128 * t)
                        pmt = sbp.tile([128, NG * GC], BF16, tag="pmt")
                        nc.vector.tensor_scalar(pmt[:], iota_c[:], cidw_sb[:, t:t + 1], None,
                                                op0=ALU.is_equal)
                        for g in range(NG):
                            gc = min(GC, NCRYS - g * GC)
                            nc.tensor.matmul(pools[g][0:gc, :], pmt[0:na, g * GC:g * GC + gc],
                                             nnat_all[0:na, 128 * t:128 * t + 128][:, 0:128],
                                             start=(t == 0), stop=(t == NBLK - 1))
                    for g in range(NG):
                        gc = min(GC, NCRYS - g * GC)
                        pev = sbp.tile([128, 128], F32, tag="pev")
                        nc.vector.tensor_copy(pev[0:gc, :], pools[g][0:gc, :])
                        nc.sync.dma_start(pool_in[g * GC:g * GC + gc, :], pev[0:gc, :])
                    if "coll" not in skip:
                        nc.gpsimd.collective_compute(
                            "AllReduce", mybir.AluOpType.add,
                            replica_groups=[list(range(NCORES))],
                            ins=[pool_in[:].opt()], outs=[pool_out[:].opt()],
                        )
                    else:
                        nc.sync.dma_start(pool_out[:], pool_in[:])

                # ---- readout (replicated) ----
                with tc.tile_pool(name=pfx + "sbr", bufs=2) as sbp, \
                     tc.tile_pool(name=pfx + "psr", bufs=2, space="PSUM") as psp:
                    for g in range(NG):
                        gc = min(GC, NCRYS - g * GC)
                        pg = sbp.tile([128, 128], F32, tag="pg")
                        nc.sync.dma_start(pg[0:gc, :], pool_out[g * GC:g * GC + gc, :])
                        mean = sbp.tile([128, 128], BF16, tag="mean")
                        nc.scalar.activation(mean[0:gc, :], pg[0:gc, :], AFT.Identity,
                                             scale=w["invccnt"][0:gc, g:g + 1])
                        ps_mt = psp.tile([128, 128], BF16, tag="pmt2")
                        nc.tensor.transpose(ps_mt[:, 0:gc], mean[0:gc, :], w["i128b"][0:gc, 0:gc])
                        meanT = sbp.tile([128, 128], BF16, tag="meanT")
                        nc.vector.tensor_copy(meanT[:, 0:gc], ps_mt[:, 0:gc])
                        ps_hr = psp.tile([128, 128], F32, tag="phr")
                        nc.tensor.matmul(ps_hr[:, 0:gc], w["readW"][:], meanT[:, 0:gc],
                                         start=True, stop=True)
                        hrT = sbp.tile([128, 128], BF16, tag="hrT")
                        nc.scalar.activation(hrT[:, 0:gc], ps_hr[:, 0:gc], act,
                                             bias=w["readb"][:])
                        ps_y = psp.tile([128, 128], F32, tag="py")
                        nc.tensor.matmul(ps_y[0:1, 0:gc], w["outW"][:], hrT[:, 0:gc],
                                         start=True, stop=True)
                        ysb = sbp.tile([1, 128], F32, tag="ysb")
                        nc.scalar.activation(ysb[0:1, 0:gc], ps_y[0:1, 0:gc], AFT.Copy,
                                             bias=meta["out_b"])
                        nc.sync.dma_start(y[0:1, g * GC:g * GC + gc], ysb[0:1, 0:gc])

            for rep in range(repeat):
                emit_once(f"r{rep}_" if repeat > 1 else "")

    nc.compile()
    return nc


def run_cores(meta, in_maps, act=AFT.Silu, sim=False):
    nc = _build(meta, act=act)
    if sim:
        from concourse.bass_interp import MultiCoreSim
        s = MultiCoreSim(nc, NCORES, trace=False)
        for k in range(NCORES):
            for nm, arr in in_maps[k].items():
                s.cores[k].tensor(nm)[:] = arr
        s.simulate(check_with_hw=False)
        return [{"y": np.array(s.cores[k].tensor("y"))} for k in range(NCORES)], None
    from concourse import bass_utils
    res = bass_utils.run_bass_kernel_spmd(nc, in_maps, core_ids=list(range(NCORES)))
    return res.results, res


def kernel(**inputs):
    cfg = dict(FULL_CFG)
    n, m = np.asarray(inputs["nbr_fea_idx"]).shape
    cfg["N"], cfg["M"] = int(n), int(m)
    cfg["AFD"] = int(np.asarray(inputs["atom_fea"]).shape[1])
    cfg["EFD"] = int(np.asarray(inputs["nbr_fea"]).shape[2])
    cfg["NCRYS"] = int(inputs["num_crystals"])
    cfg["L"] = int(np.asarray(inputs["eW1"]).shape[0])
    meta, in_maps = _prep(inputs, cfg)
    results, _ = run_cores(meta, in_maps)
    return np.asarray(results[0]["y"], np.float32).reshape(cfg["NCRYS"], 1)
